# revision 8
# baseline (speedup 1.0000x reference)
"""Trainium2 Bass kernel for a 2-layer GCN (EnhancedHockeyGNN) — v4.

v4 = v3 + (a) gathers spread over 4 SWDGE queues (Q7 core pairs run them in
parallel, ~3.4x); (b) both layers merged into ONE NEFF: the xs2 chunk tables
stay on-device (no 25.7MB copy-out, no second launch), and layer-2 tables are
loaded into the same SBUF tiles as layer-1's after the last layer-1 gather.

Changes vs v2:
  - Message tables are split into NCHUNK per-chunk DRAM tables (< 32768 rows
    each) so the gather can use the GPSIMD dma_gather custom instruction with
    int16 indices: ONE instruction per (cohort of COH groups, chunk) instead
    of one per 128-edge tile — ~10x less Pool-engine SWDGE time.
  - Edges of each group are bucketed by src chunk; tiles are per (group,
    chunk bucket), so layer-1 gathers of chunk q only wait on AllGather
    chunk q (overlaps the AG1 serial phase).
  - Layer-1 AllGather outputs are Internal addr_space="Shared" (single
    writer per chunk table) for the fast HBM-HBM collective path.
  - One-hot is_equal has the contiguous iota as in0 (double-pump port 0).
"""
import math

import numpy as np

# ---------------------------------------------------------------- constants
N = 100000
F_IN = 128
H = 128
NC = 8
SHARD = 12544            # multiple of 128; 8 * 12544 = 100352 >= N
NPAD = NC * SHARD
GROUP_EDGES = 2048       # edge budget per group
GROUP_DSTS = 128         # max dst nodes per group (PSUM partition dim)
NCHUNK = 4               # AllGather chunks == src buckets (int16 idx limit)
COH = 4                  # groups per gather cohort
EPS = 1e-5

_CACHE = {}


def _chunks(n, k):
    k = min(k, n)
    base, rem = n // k, n % k
    out, lo = [], 0
    for i in range(k):
        hi = lo + base + (1 if i < rem else 0)
        out.append((lo, hi))
        lo = hi
    return out


# ---------------------------------------------------------------- host prep
def _bin_pack(counts, G):
    order = np.argsort(-counts, kind="stable")
    bin_edges = np.zeros(G, dtype=np.int64)
    bin_nodes = np.zeros(G, dtype=np.int64)
    group_of = np.full(counts.shape[0], -1, dtype=np.int32)
    pos_in_group = np.full(counts.shape[0], -1, dtype=np.int32)
    for d in order:
        c = counts[d]
        placed = False
        for b in range(G):
            if bin_edges[b] + c <= GROUP_EDGES and bin_nodes[b] < GROUP_DSTS:
                group_of[d] = b
                pos_in_group[d] = bin_nodes[b]
                bin_edges[b] += c
                bin_nodes[b] += 1
                placed = True
                break
        if not placed:
            return None
    return group_of, pos_in_group


def _wrap_idx16(idx_flat):
    """[n] int16 -> [128, n//16] wrapped (i -> [i%16, i//16]) + replicated."""
    n = idx_flat.shape[0]
    assert n % 16 == 0
    w = idx_flat.reshape(n // 16, 16).T            # [16, cols]
    return np.tile(w, (8, 1)).copy()               # [128, cols]


def _prepare(x, edge_index, cfg):
    n, npad, shard, nc = cfg["N"], cfg["NPAD"], cfg["SHARD"], cfg["NC"]
    ge = cfg["GROUP_EDGES"]
    nchunk = cfg["NCHUNK"]
    coh = cfg["COH"]

    src = np.asarray(edge_index[0], dtype=np.int64)
    dst = np.asarray(edge_index[1], dtype=np.int64)
    deg = np.bincount(dst, minlength=n).astype(np.float64) + 1.0
    dinv = (1.0 / np.sqrt(deg)).astype(np.float32)
    dinv_pad_full = np.ones(npad, dtype=np.float32)
    dinv_pad_full[:n] = dinv

    sall = np.concatenate([src, np.arange(n, dtype=np.int64)])
    dall = np.concatenate([dst, np.arange(n, dtype=np.int64)])
    owner = dall // shard

    Es = [int((owner == c).sum()) for c in range(nc)]
    G = max(int(math.ceil(e / ge)) for e in Es)
    while True:
        packs = []
        ok = True
        for c in range(nc):
            m = owner == c
            d0 = (dall[m] - c * shard).astype(np.int64)
            counts = np.bincount(d0, minlength=shard)
            r = _bin_pack(counts, G)
            if r is None:
                ok = False
                break
            packs.append((r[0], r[1], d0, sall[m]))
        if ok:
            break
        G += 1

    ntile_nat = shard // 128

    # ----- chunk layouts
    ch_a = _chunks(ntile_nat, nchunk)          # layer-1 src buckets (nat tiles)
    ch_d = _chunks(G, nchunk)                  # layer-2 src buckets (groups)
    rows_a = [nc * (hi - lo) * 128 for lo, hi in ch_a]
    rows_d = [nc * (hi - lo) * 128 for lo, hi in ch_d]
    assert max(rows_a + rows_d) <= 32767 + 1

    nodes = np.arange(npad, dtype=np.int64)
    c_of = nodes // shard
    loc = nodes % shard
    j_of = loc // 128
    p_of = loc % 128
    tile_q = np.zeros(ntile_nat, dtype=np.int64)
    for q, (lo, hi) in enumerate(ch_a):
        tile_q[lo:hi] = q
    qa_of = tile_q[j_of]                        # layer-1 chunk of node
    lo_a = np.array([lo for lo, hi in ch_a], dtype=np.int64)[qa_of]
    nrows_a = np.array([hi - lo for lo, hi in ch_a], dtype=np.int64)[qa_of]
    row1_in_chunk = c_of * nrows_a * 128 + (j_of - lo_a) * 128 + p_of

    g_q = np.zeros(G, dtype=np.int64)
    for q, (lo, hi) in enumerate(ch_d):
        g_q[lo:hi] = q

    row2_in_chunk = np.zeros(npad, dtype=np.int64)
    qb_node = np.zeros(npad, dtype=np.int64)
    pad_cji = np.zeros((npad, 3), dtype=np.int64)
    for c in range(nc):
        group_of, pos, _, _ = packs[c]
        g64 = group_of.astype(np.int64)
        p64 = pos.astype(np.int64)
        q = g_q[g64]
        lo = np.array([l for l, _ in ch_d], dtype=np.int64)[q]
        hi = np.array([h_ for _, h_ in ch_d], dtype=np.int64)[q]
        rows = c * (hi - lo) * 128 + (g64 - lo) * 128 + p64
        row2_in_chunk[c * shard:(c + 1) * shard] = rows
        qb_node[c * shard:(c + 1) * shard] = q
        pad_cji[c * shard:(c + 1) * shard, 0] = c
        pad_cji[c * shard:(c + 1) * shard, 1] = g64
        pad_cji[c * shard:(c + 1) * shard, 2] = p64

    ncoh = (G + coh - 1) // coh

    def collect_layer(d0o, so, gstart, gend, layer):
        """Per (g, q) edge (row, dst) lists for one core/layer."""
        if layer == 1:
            q_of_edge = qa_of[so]
            row_of_edge = row1_in_chunk[so]
        else:
            q_of_edge = qb_node[so]
            row_of_edge = row2_in_chunk[so]
        edge_lists = {}
        for g in range(G):
            a, b = int(gstart[g]), int(gend[g])
            qe = q_of_edge[a:b]
            for q in range(nchunk):
                m = qe == q
                edge_lists[(g, q)] = (row_of_edge[a:b][m], d0o[a:b][m])
        return edge_lists

    def emit_layer(edge_lists, pos, T_gq):
        """Build tables for one core given the COMMON tile counts T_gq.

        call_meta[hcoh] is a list of (q, col_lo, ncols, ntiles, buf_off)
        sub-calls, each gathering <= NIDX_CAP rows.
        """
        cap_tiles = cfg.get("NIDX_CAP", 2048) // 128
        idx_cols = []
        call_meta = []
        msg_pos = [[] for _ in range(G)]
        dloc_cols = []
        oh_tiles = [int(T_gq[g].sum()) for g in range(G)]
        col_base = 0
        for hcoh in range(ncoh):
            gs = range(hcoh * coh, min((hcoh + 1) * coh, G))
            meta_h = []
            buf_off = 0
            for q in range(nchunk):
                tiles_q = 0
                idx_call = []
                for g in gs:
                    rows_e, _ = edge_lists[(g, q)]
                    T = int(T_gq[g, q])
                    assert rows_e.shape[0] <= T * 128
                    padded = np.zeros(T * 128, dtype=np.int16)
                    padded[: rows_e.shape[0]] = rows_e.astype(np.int16)
                    idx_call.append(padded)
                    for i in range(T):
                        msg_pos[g].append(buf_off + tiles_q + i)
                    tiles_q += T
                if tiles_q == 0:
                    continue
                flat = np.concatenate(idx_call)
                # split into sub-calls of <= cap_tiles tiles
                t0 = 0
                while t0 < tiles_q:
                    tpiece = min(cap_tiles, tiles_q - t0)
                    piece = flat[t0 * 128:(t0 + tpiece) * 128]
                    idx_cols.append(_wrap_idx16(piece))
                    ncols = piece.shape[0] // 16
                    meta_h.append((q, col_base, ncols, tpiece, buf_off + t0))
                    col_base += ncols
                    t0 += tpiece
                buf_off += tiles_q
            call_meta.append(meta_h)

        for g in range(G):
            for q in range(nchunk):
                _, d_e = edge_lists[(g, q)]
                T = int(T_gq[g, q])
                dl = np.full(T * 128, 300.0, dtype=np.float16)
                dl[: d_e.shape[0]] = pos[d_e]
                dloc_cols.append(dl.reshape(T, 128).T)   # [128, T]

        idx16 = np.concatenate(idx_cols, axis=1) if idx_cols else \
            np.zeros((128, 0), dtype=np.int16)
        dloc = np.concatenate(dloc_cols, axis=1)         # [128, T_total]
        return dict(idx16=idx16, dloc=dloc, call_meta=call_meta,
                    msg_pos=msg_pos, oh_tiles=oh_tiles,
                    T_total=dloc.shape[1])

    # pass 1: per-core edge lists; common (max) tile counts
    core_misc = []
    T1_gq = np.zeros((G, nchunk), dtype=np.int64)
    T2_gq = np.zeros((G, nchunk), dtype=np.int64)
    for c in range(nc):
        group_of, pos, d0, s_nodes = packs[c]
        g_of_edge = group_of[d0]
        order = np.argsort(g_of_edge, kind="stable")
        d0o, so = d0[order], s_nodes[order]
        go = g_of_edge[order]
        gstart = np.searchsorted(go, np.arange(G))
        gend = np.searchsorted(go, np.arange(G) + 1)
        el1 = collect_layer(d0o, so, gstart, gend, 1)
        el2 = collect_layer(d0o, so, gstart, gend, 2)
        for g in range(G):
            for q in range(nchunk):
                T1_gq[g, q] = max(T1_gq[g, q],
                                  (el1[(g, q)][0].shape[0] + 127) // 128)
                T2_gq[g, q] = max(T2_gq[g, q],
                                  (el2[(g, q)][0].shape[0] + 127) // 128)
        core_misc.append((el1, el2, pos, group_of))

    per_core = []
    for c in range(nc):
        el1, el2, pos, group_of = core_misc[c]
        L1 = emit_layer(el1, pos, T1_gq)
        L2 = emit_layer(el2, pos, T2_gq)

        jj = np.arange(shard)
        dinv_nat = dinv_pad_full[c * shard + jj].reshape(shard // 128, 128).T.copy()
        xs_shape = np.zeros((shard, x.shape[1]), dtype=np.float32)
        lo, hi = c * shard, min((c + 1) * shard, n)
        xs_shape[: hi - lo] = x[lo:hi]
        xT = np.ascontiguousarray(xs_shape.T).astype(np.float16)
        inv_nodes = np.full(G * 128, -1, dtype=np.int64)
        inv_nodes[group_of.astype(np.int64) * 128 + pos.astype(np.int64)] = \
            np.arange(shard)
        valid = inv_nodes >= 0
        vals = np.zeros(G * 128, dtype=np.float32)
        vals[valid] = dinv_pad_full[c * shard + inv_nodes[valid]]
        dinv_padlay = vals.reshape(G, 128).T.copy()
        ddrow = np.broadcast_to(vals.astype(np.float16)[None, :],
                                (128, G * 128)).copy()
        per_core.append(dict(L1=L1, L2=L2, dinv_nat=dinv_nat,
                             dinv_padlay=dinv_padlay, ddrow=ddrow, xT=xT))
    meta = dict(ch_a=ch_a, ch_d=ch_d, pad_cji=pad_cji,
                rows_a=rows_a, rows_d=rows_d)
    return per_core, meta, G


def _fold_bn(gamma, beta, mean, var, b):
    s = (gamma / np.sqrt(var + EPS)).astype(np.float32)
    t = ((b - mean) * s + beta).astype(np.float32)
    return s.reshape(H, 1), t.reshape(H, 1)


# ---------------------------------------------------------------- bass build
def _build(cfg, G, part, meta, L):
    import concourse.bacc as bacc
    import concourse.bass as bass
    import concourse.mybir as mybir
    import concourse.tile as tile

    fp32 = mybir.dt.float32
    fp16 = mybir.dt.float16
    i16 = mybir.dt.int16
    AF = mybir.ActivationFunctionType

    nc_ = cfg["NC"]
    shard = cfg["SHARD"]
    ntile_nat = shard // 128
    h = cfg["H"]
    fin = cfg["F_IN"]
    nchunk = cfg["NCHUNK"]
    coh = cfg["COH"]
    ch_a = meta["ch_a"]
    ch_d = meta["ch_d"]
    rows_a = meta["rows_a"]
    rows_d = meta["rows_d"]

    T_total = L["T_total"]
    call_meta = L["call_meta"]
    msg_pos = L["msg_pos"]
    oh_tiles = L["oh_tiles"]
    Tg_max = max(oh_tiles)
    ncoh = len(call_meta)
    idx_cols_total = L["idx16"].shape[1]

    nc = bacc.Bacc(None, target_bir_lowering=False, debug=False, num_devices=nc_)

    iota_in = nc.dram_tensor("iota", [128, Tg_max * 128], fp16,
                             kind="ExternalInput")
    dloc_in = nc.dram_tensor("dloc", [128, T_total], fp16, kind="ExternalInput")
    ddrow_in = nc.dram_tensor("ddrow", [128, G * 128], fp16,
                              kind="ExternalInput")
    idx_in = nc.dram_tensor("idx16", [128, idx_cols_total], i16,
                            kind="ExternalInput")

    if part == "a":
        xT_in = nc.dram_tensor("xT", [fin, shard], fp16, kind="ExternalInput")
        w1_in = nc.dram_tensor("W1", [fin, h], fp16, kind="ExternalInput")
        w2_in = nc.dram_tensor("W2", [h, h], fp16, kind="ExternalInput")
        s1_in = nc.dram_tensor("s1", [h, 1], fp32, kind="ExternalInput")
        t1_in = nc.dram_tensor("t1", [h, 1], fp32, kind="ExternalInput")
        dn_in = nc.dram_tensor("dinv_nat", [128, ntile_nat], fp32,
                               kind="ExternalInput")
        dp_in = nc.dram_tensor("dinv_padlay", [128, G], fp32,
                               kind="ExternalInput")
        out_xs2 = [nc.dram_tensor(f"xs2_out_{q}", [rows_d[q], h], fp16,
                                  kind="ExternalOutput")
                   for q in range(nchunk)]
    else:
        xs2_in = [nc.dram_tensor(f"xs2_in_{q}", [rows_d[q], h], fp16,
                                 kind="ExternalInput")
                  for q in range(nchunk)]
        wf_in = nc.dram_tensor("Wf", [h, 2], fp16, kind="ExternalInput")
        bf_in = nc.dram_tensor("bf_rep", [128, 2], fp32, kind="ExternalInput")
        s2_in = nc.dram_tensor("s2", [h, 1], fp32, kind="ExternalInput")
        t2_in = nc.dram_tensor("t2", [h, 1], fp32, kind="ExternalInput")
        out_lp = nc.dram_tensor("logp", [128, 2 * G], fp32, kind="ExternalOutput")

    with tile.TileContext(nc) as tc:
        with (
            tc.tile_pool(name="res", bufs=1) as res,
            tc.tile_pool(name="big", bufs=1) as big,
            tc.tile_pool(name="stream", bufs=1) as st,
            tc.tile_pool(name="ps", bufs=1, space="PSUM") as ps,
            tc.tile_pool(name="dram", bufs=1, space="DRAM") as dram,
        ):
            iota_t = res.tile([128, Tg_max, 128], fp16)
            dloc_t = res.tile([128, T_total], fp16)
            ddrow_t = res.tile([128, G * 128], fp16)
            idx_t = res.tile([128, idx_cols_total], i16)
            nc.sync.dma_start(out=iota_t[:],
                              in_=iota_in[:].rearrange("p (k d) -> p k d", d=128))
            nc.sync.dma_start(out=dloc_t[:], in_=dloc_in[:])
            nc.sync.dma_start(out=ddrow_t[:], in_=ddrow_in[:])
            nc.sync.dma_start(out=idx_t[:], in_=idx_in[:])

            def edge_layer(tables, s_t, t_t, post_group):
                """tables: list of nchunk DRAM APs (chunk message tables)."""
                ohcol = [0] * G
                acc = 0
                for g in range(G):
                    ohcol[g] = acc
                    acc += oh_tiles[g]
                for hcoh in range(ncoh):
                    gs = list(range(hcoh * coh, min((hcoh + 1) * coh, G)))
                    T_h = sum(oh_tiles[g] for g in gs)
                    msg = st.tile([128, T_h, h], fp16, name="msg", tag="msg",
                                  bufs=2)
                    for (q, col_lo, ncols, ntq, off) in call_meta[hcoh]:
                        nidx = ntq * 128
                        nc.gpsimd.dma_gather(
                            msg[:, off:off + ntq, :],
                            tables[q],
                            idx_t[:, col_lo:col_lo + ncols],
                            nidx,
                            nidx,
                            h,
                        )
                    for g in gs:
                        Tg = oh_tiles[g]
                        oh = st.tile([128, Tg_max, 128], fp16, name="oh",
                                     tag="oh", bufs=3)
                        nc.vector.tensor_tensor(
                            out=oh[:, :Tg, :],
                            in0=iota_t[:, :Tg, :],
                            in1=dloc_t[:, ohcol[g]:ohcol[g] + Tg]
                                .to_broadcast([128, Tg, 128]),
                            op=mybir.AluOpType.is_equal,
                        )
                        pg = ps.tile([h, 128], fp32, name="pg", tag="pg",
                                     bufs=4)
                        for i, tp in enumerate(msg_pos[g]):
                            nc.tensor.matmul(pg[:], msg[:, tp, :],
                                             oh[:, i, :],
                                             start=(i == 0),
                                             stop=(i == Tg - 1))
                        tmp = st.tile([h, 128], fp32, name="tmp", tag="tmp",
                                      bufs=4)
                        nc.vector.tensor_tensor(
                            out=tmp[:], in0=pg[:],
                            in1=ddrow_t[:, g * 128:(g + 1) * 128],
                            op=mybir.AluOpType.mult,
                        )
                        hblk = st.tile([h, 128], fp16, name="hblk",
                                       tag="hblk", bufs=4)
                        nc.scalar.activation(
                            out=hblk[:], in_=tmp[:],
                            func=AF.Relu, bias=t_t[:], scale=s_t[:],
                        )
                        post_group(g, hblk)

            if part == "a":
                w1_t = res.tile([fin, h], fp16)
                w2_t = res.tile([h, h], fp16)
                s1_t = res.tile([h, 1], fp32)
                t1_t = res.tile([h, 1], fp32)
                dn_t = res.tile([128, ntile_nat], fp32)
                dp_t = res.tile([128, G], fp32)
                for t_, i_ in ((w1_t, w1_in), (w2_t, w2_in), (s1_t, s1_in),
                               (t1_t, t1_in), (dn_t, dn_in), (dp_t, dp_in)):
                    nc.sync.dma_start(out=t_[:], in_=i_[:])

                xs1_shard = dram.tile([shard, h], fp16)
                xs1_q = [dram.tile([rows_a[q], h], fp16, addr_space="Shared",
                                   name=f"xs1q{q}")
                         for q in range(nchunk)]

                # ---- stage A: xs1 tiles, staged + AllGather'd per chunk
                xsb = big.tile([128, ntile_nat * 128], fp16, name="xsb",
                               tag="big_a")
                for q, (lo, hi) in enumerate(ch_a):
                    for j in range(lo, hi):
                        lhsT = st.tile([128, 128], fp16, name="xTt",
                                       tag="lhsT", bufs=4)
                        nc.sync.dma_start(
                            out=lhsT[:], in_=xT_in[:, j * 128:(j + 1) * 128])
                        pxs = ps.tile([128, h], fp32, name="pxs", tag="pxs",
                                      bufs=2)
                        nc.tensor.matmul(pxs[:], lhsT[:], w1_t[:], start=True,
                                         stop=True)
                        nc.vector.tensor_scalar(
                            out=xsb[:, j * 128:(j + 1) * 128], in0=pxs[:],
                            scalar1=dn_t[:, j:j + 1], scalar2=None,
                            op0=mybir.AluOpType.mult)
                    rows = hi - lo
                    dest = bass.AP(xs1_shard[:].tensor, lo * 128 * h,
                                   [[h, 128], [128 * h, rows], [1, h]])
                    nc.sync.dma_start(out=dest, in_=xsb[:].rearrange(
                        "p (j f) -> p j f", f=h)[:, lo:hi, :])
                    nc.gpsimd.collective_compute(
                        "AllGather", mybir.AluOpType.bypass,
                        replica_groups=[list(range(nc_))],
                        ins=[xs1_shard[lo * 128:hi * 128, :].opt()],
                        outs=[xs1_q[q][:].opt()],
                    )

                # ---- layer 1 with interleaved xs2 production + AG2
                xs2_shard = dram.tile([G * 128, h], fp16)
                xs2q_int = [dram.tile([rows_d[q], h], fp16,
                                      addr_space="Shared", name=f"xs2qi{q}")
                            for q in range(nchunk)]
                xs2b = big.tile([128, G * 128], fp16, name="xs2b", tag="big_c")
                g_last = {hi - 1: q for q, (lo, hi) in enumerate(ch_d)}

                def post_group(g, hblk):
                    pxs = ps.tile([128, h], fp32, name="pxs2", tag="pxs",
                                  bufs=2)
                    nc.tensor.matmul(pxs[:], hblk[:], w2_t[:], start=True,
                                     stop=True)
                    nc.vector.tensor_scalar(
                        out=xs2b[:, g * 128:(g + 1) * 128], in0=pxs[:],
                        scalar1=dp_t[:, g:g + 1], scalar2=None,
                        op0=mybir.AluOpType.mult)
                    if g in g_last:
                        q = g_last[g]
                        lo, hi = ch_d[q]
                        rows = hi - lo
                        dest = bass.AP(xs2_shard[:].tensor, lo * 128 * h,
                                       [[h, 128], [128 * h, rows], [1, h]])
                        nc.sync.dma_start(out=dest, in_=xs2b[:].rearrange(
                            "p (j f) -> p j f", f=h)[:, lo:hi, :])
                        nc.gpsimd.collective_compute(
                            "AllGather", mybir.AluOpType.bypass,
                            replica_groups=[list(range(nc_))],
                            ins=[xs2_shard[lo * 128:hi * 128, :].opt()],
                            outs=[xs2q_int[q][:].opt()],
                        )
                        nc.sync.dma_start(out=out_xs2[q][:],
                                          in_=xs2q_int[q][:])

                edge_layer([t[:] for t in xs1_q], s1_t, t1_t, post_group)
            else:
                wf_t = res.tile([h, 2], fp16)
                bf_t = res.tile([128, 2], fp32)
                s2_t = res.tile([h, 1], fp32)
                t2_t = res.tile([h, 1], fp32)
                for t_, i_ in ((wf_t, wf_in), (bf_t, bf_in), (s2_t, s2_in),
                               (t2_t, t2_in)):
                    nc.sync.dma_start(out=t_[:], in_=i_[:])

                lg = res.tile([128, 2 * G], fp32)

                def post_group_b(g, hblk):
                    plg = ps.tile([128, 2], fp32, name="plg", tag="plg",
                                  bufs=2)
                    nc.tensor.matmul(plg[:], hblk[:], wf_t[:], start=True,
                                     stop=True)
                    nc.vector.tensor_add(out=lg[:, 2 * g:2 * g + 2],
                                         in0=plg[:], in1=bf_t[:])

                edge_layer([t[:] for t in xs2_in], s2_t, t2_t, post_group_b)

                def strided(base, start):
                    a = base[:]
                    return bass.AP(a.tensor, a.offset + start,
                                   [a.ap[0], [2, G]])

                z0, z1 = strided(lg, 0), strided(lg, 1)
                mx = res.tile([128, G], fp32)
                nc.vector.tensor_tensor(out=mx[:], in0=z0, in1=z1,
                                        op=mybir.AluOpType.max)
                sm0 = res.tile([128, G], fp32)
                sm1 = res.tile([128, G], fp32)
                nc.vector.tensor_sub(out=sm0[:], in0=z0, in1=mx[:])
                nc.vector.tensor_sub(out=sm1[:], in0=z1, in1=mx[:])
                e0 = res.tile([128, G], fp32)
                e1 = res.tile([128, G], fp32)
                nc.scalar.activation(out=e0[:], in_=sm0[:], func=AF.Exp)
                nc.scalar.activation(out=e1[:], in_=sm1[:], func=AF.Exp)
                se = res.tile([128, G], fp32)
                nc.vector.tensor_add(out=se[:], in0=e0[:], in1=e1[:])
                ls = res.tile([128, G], fp32)
                nc.scalar.activation(out=ls[:], in_=se[:], func=AF.Ln)
                nc.vector.tensor_sub(out=sm0[:], in0=sm0[:], in1=ls[:])
                nc.vector.tensor_sub(out=sm1[:], in0=sm1[:], in1=ls[:])
                lpo = res.tile([128, 2 * G], fp32)
                nc.vector.tensor_copy(out=strided(lpo, 0), in_=sm0[:])
                nc.vector.tensor_copy(out=strided(lpo, 1), in_=sm1[:])
                nc.sync.dma_start(out=out_lp[:], in_=lpo[:])

    nc.compile()
    return nc


# ---------------------------------------------------------------- main entry
def _run(x, edge_index, game_indices,
         W1, b1, g1, be1, m1, v1, W2, b2, g2, be2, m2, v2, Wf, bf,
         trace=False, cfg=None):
    from concourse import bass_utils

    if cfg is None:
        cfg = dict(N=N, NPAD=NPAD, SHARD=SHARD, NC=NC, GROUP_EDGES=GROUP_EDGES,
                   H=H, F_IN=F_IN, NCHUNK=NCHUNK, COH=COH, NIDX_CAP=1024)

    x = np.asarray(x, dtype=np.float32)
    key = ("prep", x.shape, int(np.asarray(edge_index)[0, 0]),
           int(np.asarray(edge_index).sum() % (1 << 31)))
    if key in _CACHE:
        per_core, meta, G = _CACHE[key]
    else:
        per_core, meta, G = _prepare(x, np.asarray(edge_index), cfg)
        _CACHE.clear()
        _CACHE[key] = (per_core, meta, G)

    # all cores share the same tile-structure *shapes* only if identical;
    # build per distinct shape signature
    def sig(L):
        return (L["T_total"], L["idx16"].shape[1],
                tuple(tuple(m) for h_ in L["call_meta"] for m in h_),
                tuple(oh for oh in L["oh_tiles"]),
                tuple(tuple(p) for p in L["msg_pos"]))

    bkey_a = ("bass_a", G, sig(per_core[0]["L1"]))
    bkey_b = ("bass_b", G, sig(per_core[0]["L2"]))
    same_a = all(sig(pc["L1"]) == sig(per_core[0]["L1"]) for pc in per_core)
    same_b = all(sig(pc["L2"]) == sig(per_core[0]["L2"]) for pc in per_core)
    assert same_a and same_b, "per-core tile structures differ; SPMD needs one"

    if bkey_a in _CACHE:
        nc_a = _CACHE[bkey_a]
    else:
        nc_a = _build(cfg, G, "a", meta, per_core[0]["L1"])
        _CACHE[bkey_a] = nc_a
    if bkey_b in _CACHE:
        nc_b = _CACHE[bkey_b]
    else:
        nc_b = _build(cfg, G, "b", meta, per_core[0]["L2"])
        _CACHE[bkey_b] = nc_b

    s1, t1 = _fold_bn(np.asarray(g1), np.asarray(be1), np.asarray(m1),
                      np.asarray(v1), np.asarray(b1))
    s2, t2 = _fold_bn(np.asarray(g2), np.asarray(be2), np.asarray(m2),
                      np.asarray(v2), np.asarray(b2))
    Tg_max1 = max(per_core[0]["L1"]["oh_tiles"])
    Tg_max2 = max(per_core[0]["L2"]["oh_tiles"])
    bf_rep = np.broadcast_to(np.asarray(bf, dtype=np.float32), (128, 2)).copy()

    ncores = cfg["NC"]
    in_maps_a = []
    for c in range(ncores):
        pc = per_core[c]
        in_maps_a.append(dict(
            xT=pc["xT"], W1=np.asarray(W1, np.float16),
            W2=np.asarray(W2, np.float16), s1=s1, t1=t1,
            iota=np.tile(np.arange(128, dtype=np.float16), (128, Tg_max1)),
            idx16=pc["L1"]["idx16"], dloc=pc["L1"]["dloc"],
            ddrow=pc["ddrow"], dinv_nat=pc["dinv_nat"],
            dinv_padlay=pc["dinv_padlay"],
        ))
    res_a = bass_utils.run_bass_kernel_spmd(
        nc_a, in_maps_a, core_ids=list(range(ncores)), trace=trace)

    in_maps_b = []
    for c in range(ncores):
        pc = per_core[c]
        m = dict(
            Wf=np.asarray(Wf, np.float16), bf_rep=bf_rep, s2=s2, t2=t2,
            iota=np.tile(np.arange(128, dtype=np.float16), (128, Tg_max2)),
            idx16=pc["L2"]["idx16"], dloc=pc["L2"]["dloc"], ddrow=pc["ddrow"],
        )
        for q in range(cfg["NCHUNK"]):
            m[f"xs2_in_{q}"] = res_a.results[c][f"xs2_out_{q}"]
        in_maps_b.append(m)
    res_b = bass_utils.run_bass_kernel_spmd(
        nc_b, in_maps_b, core_ids=list(range(ncores)), trace=trace)

    class _Res:
        pass

    res = _Res()
    res.results = res_b.results
    res.exec_time_ns = ((res_a.exec_time_ns or 0) + (res_b.exec_time_ns or 0)) \
        if (res_a.exec_time_ns or res_b.exec_time_ns) else None
    res.parts = (res_a, res_b)

    gi = np.asarray(game_indices, dtype=np.int64)
    cji = meta["pad_cji"][gi]
    lp = np.stack([res_b.results[c]["logp"] for c in range(ncores)])
    out = np.empty((gi.shape[0], 2), dtype=np.float32)
    out[:, 0] = lp[cji[:, 0], cji[:, 2], 2 * cji[:, 1]]
    out[:, 1] = lp[cji[:, 0], cji[:, 2], 2 * cji[:, 1] + 1]
    return out, res


def kernel(**inputs):
    out, _ = _run(**inputs)
    return out


def kernel_profiled(**inputs):
    out, res = _run(**inputs, trace=True)
    return out, res


# revision 9
# speedup vs baseline: 1.3376x; 1.3376x over previous
"""Trainium2 Bass kernel for a 2-layer GCN (EnhancedHockeyGNN) — v4.

v4 = v3 + (a) gathers spread over 4 SWDGE queues (Q7 core pairs run them in
parallel, ~3.4x); (b) both layers merged into ONE NEFF: the xs2 chunk tables
stay on-device (no 25.7MB copy-out, no second launch), and layer-2 tables are
loaded into the same SBUF tiles as layer-1's after the last layer-1 gather.

Changes vs v2:
  - Message tables are split into NCHUNK per-chunk DRAM tables (< 32768 rows
    each) so the gather can use the GPSIMD dma_gather custom instruction with
    int16 indices: ONE instruction per (cohort of COH groups, chunk) instead
    of one per 128-edge tile — ~10x less Pool-engine SWDGE time.
  - Edges of each group are bucketed by src chunk; tiles are per (group,
    chunk bucket), so layer-1 gathers of chunk q only wait on AllGather
    chunk q (overlaps the AG1 serial phase).
  - Layer-1 AllGather outputs are Internal addr_space="Shared" (single
    writer per chunk table) for the fast HBM-HBM collective path.
  - One-hot is_equal has the contiguous iota as in0 (double-pump port 0).
"""
import math

import numpy as np

# ---------------------------------------------------------------- constants
N = 100000
F_IN = 128
H = 128
NC = 8
SHARD = 12544            # multiple of 128; 8 * 12544 = 100352 >= N
NPAD = NC * SHARD
GROUP_EDGES = 2048       # edge budget per group
GROUP_DSTS = 128         # max dst nodes per group (PSUM partition dim)
NCHUNK = 4               # AllGather chunks == src buckets (int16 idx limit)
COH = 4                  # groups per gather cohort
EPS = 1e-5

_CACHE = {}


def _chunks(n, k):
    k = min(k, n)
    base, rem = n // k, n % k
    out, lo = [], 0
    for i in range(k):
        hi = lo + base + (1 if i < rem else 0)
        out.append((lo, hi))
        lo = hi
    return out


# ---------------------------------------------------------------- host prep
def _bin_pack(counts, G):
    order = np.argsort(-counts, kind="stable")
    bin_edges = np.zeros(G, dtype=np.int64)
    bin_nodes = np.zeros(G, dtype=np.int64)
    group_of = np.full(counts.shape[0], -1, dtype=np.int32)
    pos_in_group = np.full(counts.shape[0], -1, dtype=np.int32)
    for d in order:
        c = counts[d]
        placed = False
        for b in range(G):
            if bin_edges[b] + c <= GROUP_EDGES and bin_nodes[b] < GROUP_DSTS:
                group_of[d] = b
                pos_in_group[d] = bin_nodes[b]
                bin_edges[b] += c
                bin_nodes[b] += 1
                placed = True
                break
        if not placed:
            return None
    return group_of, pos_in_group


def _wrap_idx16(idx_flat):
    """[n] int16 -> [128, n//16] wrapped (i -> [i%16, i//16]) + replicated."""
    n = idx_flat.shape[0]
    assert n % 16 == 0
    w = idx_flat.reshape(n // 16, 16).T            # [16, cols]
    return np.tile(w, (8, 1)).copy()               # [128, cols]


def _prepare(x, edge_index, cfg):
    n, npad, shard, nc = cfg["N"], cfg["NPAD"], cfg["SHARD"], cfg["NC"]
    ge = cfg["GROUP_EDGES"]
    nchunk = cfg["NCHUNK"]
    coh = cfg["COH"]

    src = np.asarray(edge_index[0], dtype=np.int64)
    dst = np.asarray(edge_index[1], dtype=np.int64)
    deg = np.bincount(dst, minlength=n).astype(np.float64) + 1.0
    dinv = (1.0 / np.sqrt(deg)).astype(np.float32)
    dinv_pad_full = np.ones(npad, dtype=np.float32)
    dinv_pad_full[:n] = dinv

    sall = np.concatenate([src, np.arange(n, dtype=np.int64)])
    dall = np.concatenate([dst, np.arange(n, dtype=np.int64)])
    owner = dall // shard

    Es = [int((owner == c).sum()) for c in range(nc)]
    G = max(int(math.ceil(e / ge)) for e in Es)
    while True:
        packs = []
        ok = True
        for c in range(nc):
            m = owner == c
            d0 = (dall[m] - c * shard).astype(np.int64)
            counts = np.bincount(d0, minlength=shard)
            r = _bin_pack(counts, G)
            if r is None:
                ok = False
                break
            packs.append((r[0], r[1], d0, sall[m]))
        if ok:
            break
        G += 1

    ntile_nat = shard // 128

    # ----- chunk layouts
    ch_a = _chunks(ntile_nat, nchunk)          # layer-1 src buckets (nat tiles)
    ch_d = _chunks(G, nchunk)                  # layer-2 src buckets (groups)
    rows_a = [nc * (hi - lo) * 128 for lo, hi in ch_a]
    rows_d = [nc * (hi - lo) * 128 for lo, hi in ch_d]
    assert max(rows_a + rows_d) <= 32767 + 1

    nodes = np.arange(npad, dtype=np.int64)
    c_of = nodes // shard
    loc = nodes % shard
    j_of = loc // 128
    p_of = loc % 128
    tile_q = np.zeros(ntile_nat, dtype=np.int64)
    for q, (lo, hi) in enumerate(ch_a):
        tile_q[lo:hi] = q
    qa_of = tile_q[j_of]                        # layer-1 chunk of node
    lo_a = np.array([lo for lo, hi in ch_a], dtype=np.int64)[qa_of]
    nrows_a = np.array([hi - lo for lo, hi in ch_a], dtype=np.int64)[qa_of]
    row1_in_chunk = c_of * nrows_a * 128 + (j_of - lo_a) * 128 + p_of

    g_q = np.zeros(G, dtype=np.int64)
    for q, (lo, hi) in enumerate(ch_d):
        g_q[lo:hi] = q

    row2_in_chunk = np.zeros(npad, dtype=np.int64)
    qb_node = np.zeros(npad, dtype=np.int64)
    pad_cji = np.zeros((npad, 3), dtype=np.int64)
    for c in range(nc):
        group_of, pos, _, _ = packs[c]
        g64 = group_of.astype(np.int64)
        p64 = pos.astype(np.int64)
        q = g_q[g64]
        lo = np.array([l for l, _ in ch_d], dtype=np.int64)[q]
        hi = np.array([h_ for _, h_ in ch_d], dtype=np.int64)[q]
        rows = c * (hi - lo) * 128 + (g64 - lo) * 128 + p64
        row2_in_chunk[c * shard:(c + 1) * shard] = rows
        qb_node[c * shard:(c + 1) * shard] = q
        pad_cji[c * shard:(c + 1) * shard, 0] = c
        pad_cji[c * shard:(c + 1) * shard, 1] = g64
        pad_cji[c * shard:(c + 1) * shard, 2] = p64

    ncoh = (G + coh - 1) // coh

    def collect_layer(d0o, so, gstart, gend, layer):
        """Per (g, q) edge (row, dst) lists for one core/layer."""
        if layer == 1:
            q_of_edge = qa_of[so]
            row_of_edge = row1_in_chunk[so]
        else:
            q_of_edge = qb_node[so]
            row_of_edge = row2_in_chunk[so]
        edge_lists = {}
        for g in range(G):
            a, b = int(gstart[g]), int(gend[g])
            qe = q_of_edge[a:b]
            for q in range(nchunk):
                m = qe == q
                edge_lists[(g, q)] = (row_of_edge[a:b][m], d0o[a:b][m])
        return edge_lists

    def emit_layer(edge_lists, pos, T_gq):
        """Build tables for one core given the COMMON tile counts T_gq.

        call_meta[hcoh] is a list of (q, col_lo, ncols, ntiles, buf_off)
        sub-calls, each gathering <= NIDX_CAP rows.
        """
        cap_tiles = cfg.get("NIDX_CAP", 2048) // 128
        idx_cols = []
        call_meta = []
        msg_pos = [[] for _ in range(G)]
        dloc_cols = []
        oh_tiles = [int(T_gq[g].sum()) for g in range(G)]
        col_base = 0
        for hcoh in range(ncoh):
            gs = range(hcoh * coh, min((hcoh + 1) * coh, G))
            meta_h = []
            buf_off = 0
            for q in range(nchunk):
                tiles_q = 0
                idx_call = []
                for g in gs:
                    rows_e, _ = edge_lists[(g, q)]
                    T = int(T_gq[g, q])
                    assert rows_e.shape[0] <= T * 128
                    padded = np.zeros(T * 128, dtype=np.int16)
                    padded[: rows_e.shape[0]] = rows_e.astype(np.int16)
                    idx_call.append(padded)
                    for i in range(T):
                        msg_pos[g].append(buf_off + tiles_q + i)
                    tiles_q += T
                if tiles_q == 0:
                    continue
                flat = np.concatenate(idx_call)
                # split into sub-calls of <= cap_tiles tiles
                t0 = 0
                while t0 < tiles_q:
                    tpiece = min(cap_tiles, tiles_q - t0)
                    piece = flat[t0 * 128:(t0 + tpiece) * 128]
                    idx_cols.append(_wrap_idx16(piece))
                    ncols = piece.shape[0] // 16
                    meta_h.append((q, col_base, ncols, tpiece, buf_off + t0))
                    col_base += ncols
                    t0 += tpiece
                buf_off += tiles_q
            call_meta.append(meta_h)

        for g in range(G):
            for q in range(nchunk):
                _, d_e = edge_lists[(g, q)]
                T = int(T_gq[g, q])
                dl = np.full(T * 128, 300.0, dtype=np.float16)
                dl[: d_e.shape[0]] = pos[d_e]
                dloc_cols.append(dl.reshape(T, 128).T)   # [128, T]

        idx16 = np.concatenate(idx_cols, axis=1) if idx_cols else \
            np.zeros((128, 0), dtype=np.int16)
        dloc = np.concatenate(dloc_cols, axis=1)         # [128, T_total]
        return dict(idx16=idx16, dloc=dloc, call_meta=call_meta,
                    msg_pos=msg_pos, oh_tiles=oh_tiles,
                    T_total=dloc.shape[1])

    # pass 1: per-core edge lists; common (max) tile counts
    core_misc = []
    T1_gq = np.zeros((G, nchunk), dtype=np.int64)
    T2_gq = np.zeros((G, nchunk), dtype=np.int64)
    for c in range(nc):
        group_of, pos, d0, s_nodes = packs[c]
        g_of_edge = group_of[d0]
        order = np.argsort(g_of_edge, kind="stable")
        d0o, so = d0[order], s_nodes[order]
        go = g_of_edge[order]
        gstart = np.searchsorted(go, np.arange(G))
        gend = np.searchsorted(go, np.arange(G) + 1)
        el1 = collect_layer(d0o, so, gstart, gend, 1)
        el2 = collect_layer(d0o, so, gstart, gend, 2)
        for g in range(G):
            for q in range(nchunk):
                T1_gq[g, q] = max(T1_gq[g, q],
                                  (el1[(g, q)][0].shape[0] + 127) // 128)
                T2_gq[g, q] = max(T2_gq[g, q],
                                  (el2[(g, q)][0].shape[0] + 127) // 128)
        core_misc.append((el1, el2, pos, group_of))

    per_core = []
    for c in range(nc):
        el1, el2, pos, group_of = core_misc[c]
        L1 = emit_layer(el1, pos, T1_gq)
        L2 = emit_layer(el2, pos, T2_gq)

        jj = np.arange(shard)
        dinv_nat = dinv_pad_full[c * shard + jj].reshape(shard // 128, 128).T.copy()
        xs_shape = np.zeros((shard, x.shape[1]), dtype=np.float32)
        lo, hi = c * shard, min((c + 1) * shard, n)
        xs_shape[: hi - lo] = x[lo:hi]
        xT = np.ascontiguousarray(xs_shape.T).astype(np.float16)
        inv_nodes = np.full(G * 128, -1, dtype=np.int64)
        inv_nodes[group_of.astype(np.int64) * 128 + pos.astype(np.int64)] = \
            np.arange(shard)
        valid = inv_nodes >= 0
        vals = np.zeros(G * 128, dtype=np.float32)
        vals[valid] = dinv_pad_full[c * shard + inv_nodes[valid]]
        dinv_padlay = vals.reshape(G, 128).T.copy()
        ddrow = np.broadcast_to(vals.astype(np.float16)[None, :],
                                (128, G * 128)).copy()
        per_core.append(dict(L1=L1, L2=L2, dinv_nat=dinv_nat,
                             dinv_padlay=dinv_padlay, ddrow=ddrow, xT=xT))
    meta = dict(ch_a=ch_a, ch_d=ch_d, pad_cji=pad_cji,
                rows_a=rows_a, rows_d=rows_d)
    return per_core, meta, G


def _fold_bn(gamma, beta, mean, var, b):
    s = (gamma / np.sqrt(var + EPS)).astype(np.float32)
    t = ((b - mean) * s + beta).astype(np.float32)
    return s.reshape(H, 1), t.reshape(H, 1)


# ---------------------------------------------------------------- bass build
def _build(cfg, G, meta, L1, L2):
    import concourse.bacc as bacc
    import concourse.bass as bass
    import concourse.mybir as mybir
    import concourse.tile as tile

    fp32 = mybir.dt.float32
    fp16 = mybir.dt.float16
    i16 = mybir.dt.int16
    AF = mybir.ActivationFunctionType

    nc_ = cfg["NC"]
    shard = cfg["SHARD"]
    ntile_nat = shard // 128
    h = cfg["H"]
    fin = cfg["F_IN"]
    nchunk = cfg["NCHUNK"]
    coh = cfg["COH"]
    ch_a = meta["ch_a"]
    ch_d = meta["ch_d"]
    rows_a = meta["rows_a"]
    rows_d = meta["rows_d"]

    Tg_max = max(max(L1["oh_tiles"]), max(L2["oh_tiles"]))
    Tt_max = max(L1["T_total"], L2["T_total"])
    cols_max = max(L1["idx16"].shape[1], L2["idx16"].shape[1])

    nc = bacc.Bacc(None, target_bir_lowering=False, debug=False,
                   num_devices=nc_, num_swdge_queues=4)

    iota_in = nc.dram_tensor("iota", [128, Tg_max * 128], fp16,
                             kind="ExternalInput")
    dloc1_in = nc.dram_tensor("dloc1", [128, L1["T_total"]], fp16,
                              kind="ExternalInput")
    dloc2_in = nc.dram_tensor("dloc2", [128, L2["T_total"]], fp16,
                              kind="ExternalInput")
    idx1_in = nc.dram_tensor("idx1", [128, L1["idx16"].shape[1]], i16,
                             kind="ExternalInput")
    idx2_in = nc.dram_tensor("idx2", [128, L2["idx16"].shape[1]], i16,
                             kind="ExternalInput")
    ddrow_in = nc.dram_tensor("ddrow", [128, G * 128], fp16,
                              kind="ExternalInput")
    xT_in = nc.dram_tensor("xT", [fin, shard], fp16, kind="ExternalInput")
    w1_in = nc.dram_tensor("W1", [fin, h], fp16, kind="ExternalInput")
    w2_in = nc.dram_tensor("W2", [h, h], fp16, kind="ExternalInput")
    wf_in = nc.dram_tensor("Wf", [h, 2], fp16, kind="ExternalInput")
    bf_in = nc.dram_tensor("bf_rep", [128, 2], fp32, kind="ExternalInput")
    s1_in = nc.dram_tensor("s1", [h, 1], fp32, kind="ExternalInput")
    t1_in = nc.dram_tensor("t1", [h, 1], fp32, kind="ExternalInput")
    s2_in = nc.dram_tensor("s2", [h, 1], fp32, kind="ExternalInput")
    t2_in = nc.dram_tensor("t2", [h, 1], fp32, kind="ExternalInput")
    dn_in = nc.dram_tensor("dinv_nat", [128, ntile_nat], fp32,
                           kind="ExternalInput")
    dp_in = nc.dram_tensor("dinv_padlay", [128, G], fp32,
                           kind="ExternalInput")
    out_lp = nc.dram_tensor("logp", [128, 2 * G], fp32, kind="ExternalOutput")

    with tile.TileContext(nc) as tc:
        with (
            tc.tile_pool(name="res", bufs=1) as res,
            tc.tile_pool(name="big", bufs=1) as big,
            tc.tile_pool(name="stream", bufs=1) as st,
            tc.tile_pool(name="ps", bufs=1, space="PSUM") as ps,
            tc.tile_pool(name="dram", bufs=1, space="DRAM") as dram,
        ):
            iota_t = res.tile([128, Tg_max, 128], fp16)
            dloc_t = res.tile([128, Tt_max], fp16)
            idx_t = res.tile([128, cols_max], i16)
            ddrow_t = res.tile([128, G * 128], fp16)
            w1_t = res.tile([fin, h], fp16)
            w2_t = res.tile([h, h], fp16)
            wf_t = res.tile([h, 2], fp16)
            bf_t = res.tile([128, 2], fp32)
            s1_t = res.tile([h, 1], fp32)
            t1_t = res.tile([h, 1], fp32)
            s2_t = res.tile([h, 1], fp32)
            t2_t = res.tile([h, 1], fp32)
            dn_t = res.tile([128, ntile_nat], fp32)
            dp_t = res.tile([128, G], fp32)
            nc.sync.dma_start(out=iota_t[:],
                              in_=iota_in[:].rearrange("p (k d) -> p k d", d=128))
            nc.sync.dma_start(out=ddrow_t[:], in_=ddrow_in[:])
            for t_, i_ in ((w1_t, w1_in), (w2_t, w2_in), (wf_t, wf_in),
                           (bf_t, bf_in), (s1_t, s1_in), (t1_t, t1_in),
                           (s2_t, s2_in), (t2_t, t2_in), (dn_t, dn_in),
                           (dp_t, dp_in)):
                nc.sync.dma_start(out=t_[:], in_=i_[:])
            nc.sync.dma_start(out=idx_t[:, :L1["idx16"].shape[1]],
                              in_=idx1_in[:])
            nc.sync.dma_start(out=dloc_t[:, :L1["T_total"]], in_=dloc1_in[:])

            def edge_layer(L, tables, s_t, t_t, post_group):
                call_meta = L["call_meta"]
                msg_pos = L["msg_pos"]
                oh_tiles = L["oh_tiles"]
                ncoh = len(call_meta)
                ohcol = [0] * G
                acc = 0
                for g in range(G):
                    ohcol[g] = acc
                    acc += oh_tiles[g]
                for hcoh in range(ncoh):
                    gs = list(range(hcoh * coh, min((hcoh + 1) * coh, G)))
                    T_h = sum(oh_tiles[g] for g in gs)
                    msg = st.tile([128, T_h, h], fp16, name="msg", tag="msg",
                                  bufs=2)
                    for (q, col_lo, ncols, ntq, off) in call_meta[hcoh]:
                        nidx = ntq * 128
                        nc.gpsimd.dma_gather(
                            msg[:, off:off + ntq, :],
                            tables[q],
                            idx_t[:, col_lo:col_lo + ncols],
                            nidx,
                            nidx,
                            h,
                            queue_num=q,
                        )
                    for g in gs:
                        Tg = oh_tiles[g]
                        oh = st.tile([128, Tg_max, 128], fp16, name="oh",
                                     tag="oh", bufs=3)
                        nc.vector.tensor_tensor(
                            out=oh[:, :Tg, :],
                            in0=iota_t[:, :Tg, :],
                            in1=dloc_t[:, ohcol[g]:ohcol[g] + Tg]
                                .to_broadcast([128, Tg, 128]),
                            op=mybir.AluOpType.is_equal,
                        )
                        pg = ps.tile([h, 128], fp32, name="pg", tag="pg",
                                     bufs=4)
                        for i, tp in enumerate(msg_pos[g]):
                            nc.tensor.matmul(pg[:], msg[:, tp, :],
                                             oh[:, i, :],
                                             start=(i == 0),
                                             stop=(i == Tg - 1))
                        tmp = st.tile([h, 128], fp32, name="tmp", tag="tmp",
                                      bufs=4)
                        nc.vector.tensor_tensor(
                            out=tmp[:], in0=pg[:],
                            in1=ddrow_t[:, g * 128:(g + 1) * 128],
                            op=mybir.AluOpType.mult,
                        )
                        hblk = st.tile([h, 128], fp16, name="hblk",
                                       tag="hblk", bufs=4)
                        nc.scalar.activation(
                            out=hblk[:], in_=tmp[:],
                            func=AF.Relu, bias=t_t[:], scale=s_t[:],
                        )
                        post_group(g, hblk)

            # ---- stage A: xs1 compute, staged + AllGather'd per chunk
            xs1_shard = dram.tile([shard, h], fp16)
            xs1_q = [dram.tile([rows_a[q], h], fp16, addr_space="Shared",
                               name=f"xs1q{q}")
                     for q in range(nchunk)]
            xsb = big.tile([128, ntile_nat * 128], fp16, name="xsb",
                           tag="big_a")
            for q, (lo, hi) in enumerate(ch_a):
                for j in range(lo, hi):
                    lhsT = st.tile([128, 128], fp16, name="xTt",
                                   tag="lhsT", bufs=4)
                    nc.sync.dma_start(
                        out=lhsT[:], in_=xT_in[:, j * 128:(j + 1) * 128])
                    pxs = ps.tile([128, h], fp32, name="pxs", tag="pxs",
                                  bufs=2)
                    nc.tensor.matmul(pxs[:], lhsT[:], w1_t[:], start=True,
                                     stop=True)
                    nc.vector.tensor_scalar(
                        out=xsb[:, j * 128:(j + 1) * 128], in0=pxs[:],
                        scalar1=dn_t[:, j:j + 1], scalar2=None,
                        op0=mybir.AluOpType.mult)
                rows = hi - lo
                dest = bass.AP(xs1_shard[:].tensor, lo * 128 * h,
                               [[h, 128], [128 * h, rows], [1, h]])
                nc.sync.dma_start(out=dest, in_=xsb[:].rearrange(
                    "p (j f) -> p j f", f=h)[:, lo:hi, :])
                nc.gpsimd.collective_compute(
                    "AllGather", mybir.AluOpType.bypass,
                    replica_groups=[list(range(nc_))],
                    ins=[xs1_shard[lo * 128:hi * 128, :].opt()],
                    outs=[xs1_q[q][:].opt()],
                )

            # ---- layer 1 with interleaved xs2 production + AG2
            xs2_shard = dram.tile([G * 128, h], fp16)
            xs2q_int = [dram.tile([rows_d[q], h], fp16,
                                  addr_space="Shared", name=f"xs2qi{q}")
                        for q in range(nchunk)]
            xs2b = big.tile([128, G * 128], fp16, name="xs2b", tag="big_c")
            g_last = {hi - 1: q for q, (lo, hi) in enumerate(ch_d)}

            def post_group_a(g, hblk):
                pxs = ps.tile([128, h], fp32, name="pxs2", tag="pxs",
                              bufs=2)
                nc.tensor.matmul(pxs[:], hblk[:], w2_t[:], start=True,
                                 stop=True)
                nc.vector.tensor_scalar(
                    out=xs2b[:, g * 128:(g + 1) * 128], in0=pxs[:],
                    scalar1=dp_t[:, g:g + 1], scalar2=None,
                    op0=mybir.AluOpType.mult)
                if g in g_last:
                    q = g_last[g]
                    lo, hi = ch_d[q]
                    rows = hi - lo
                    dest = bass.AP(xs2_shard[:].tensor, lo * 128 * h,
                                   [[h, 128], [128 * h, rows], [1, h]])
                    nc.sync.dma_start(out=dest, in_=xs2b[:].rearrange(
                        "p (j f) -> p j f", f=h)[:, lo:hi, :])
                    nc.gpsimd.collective_compute(
                        "AllGather", mybir.AluOpType.bypass,
                        replica_groups=[list(range(nc_))],
                        ins=[xs2_shard[lo * 128:hi * 128, :].opt()],
                        outs=[xs2q_int[q][:].opt()],
                    )

            edge_layer(L1, [t[:] for t in xs1_q], s1_t, t1_t, post_group_a)

            # ---- swap in layer-2 tables (reuse the same SBUF tiles)
            nc.sync.dma_start(out=idx_t[:, :L2["idx16"].shape[1]],
                              in_=idx2_in[:])
            nc.sync.dma_start(out=dloc_t[:, :L2["T_total"]], in_=dloc2_in[:])

            lg = res.tile([128, 2 * G], fp32)

            def post_group_b(g, hblk):
                plg = ps.tile([128, 2], fp32, name="plg", tag="plg",
                              bufs=2)
                nc.tensor.matmul(plg[:], hblk[:], wf_t[:], start=True,
                                 stop=True)
                nc.vector.tensor_add(out=lg[:, 2 * g:2 * g + 2],
                                     in0=plg[:], in1=bf_t[:])

            edge_layer(L2, [t[:] for t in xs2q_int], s2_t, t2_t, post_group_b)

            def strided(base, start):
                a = base[:]
                return bass.AP(a.tensor, a.offset + start,
                               [a.ap[0], [2, G]])

            z0, z1 = strided(lg, 0), strided(lg, 1)
            mx = res.tile([128, G], fp32)
            nc.vector.tensor_tensor(out=mx[:], in0=z0, in1=z1,
                                    op=mybir.AluOpType.max)
            sm0 = res.tile([128, G], fp32)
            sm1 = res.tile([128, G], fp32)
            nc.vector.tensor_sub(out=sm0[:], in0=z0, in1=mx[:])
            nc.vector.tensor_sub(out=sm1[:], in0=z1, in1=mx[:])
            e0 = res.tile([128, G], fp32)
            e1 = res.tile([128, G], fp32)
            nc.scalar.activation(out=e0[:], in_=sm0[:], func=AF.Exp)
            nc.scalar.activation(out=e1[:], in_=sm1[:], func=AF.Exp)
            se = res.tile([128, G], fp32)
            nc.vector.tensor_add(out=se[:], in0=e0[:], in1=e1[:])
            ls = res.tile([128, G], fp32)
            nc.scalar.activation(out=ls[:], in_=se[:], func=AF.Ln)
            nc.vector.tensor_sub(out=sm0[:], in0=sm0[:], in1=ls[:])
            nc.vector.tensor_sub(out=sm1[:], in0=sm1[:], in1=ls[:])
            lpo = res.tile([128, 2 * G], fp32)
            nc.vector.tensor_copy(out=strided(lpo, 0), in_=sm0[:])
            nc.vector.tensor_copy(out=strided(lpo, 1), in_=sm1[:])
            nc.sync.dma_start(out=out_lp[:], in_=lpo[:])

    nc.compile()
    return nc


# ---------------------------------------------------------------- main entry
def _run(x, edge_index, game_indices,
         W1, b1, g1, be1, m1, v1, W2, b2, g2, be2, m2, v2, Wf, bf,
         trace=False, cfg=None):
    from concourse import bass_utils

    if cfg is None:
        cfg = dict(N=N, NPAD=NPAD, SHARD=SHARD, NC=NC, GROUP_EDGES=GROUP_EDGES,
                   H=H, F_IN=F_IN, NCHUNK=NCHUNK, COH=COH, NIDX_CAP=1024)

    x = np.asarray(x, dtype=np.float32)
    key = ("prep", x.shape, int(np.asarray(edge_index)[0, 0]),
           int(np.asarray(edge_index).sum() % (1 << 31)))
    if key in _CACHE:
        per_core, meta, G = _CACHE[key]
    else:
        per_core, meta, G = _prepare(x, np.asarray(edge_index), cfg)
        _CACHE.clear()
        _CACHE[key] = (per_core, meta, G)

    def sig(L):
        return (L["T_total"], L["idx16"].shape[1],
                tuple(tuple(m) for h_ in L["call_meta"] for m in h_))

    assert all(sig(pc["L1"]) == sig(per_core[0]["L1"]) for pc in per_core)
    assert all(sig(pc["L2"]) == sig(per_core[0]["L2"]) for pc in per_core)

    bkey = ("bass", G, sig(per_core[0]["L1"]), sig(per_core[0]["L2"]))
    if bkey in _CACHE:
        nc_m = _CACHE[bkey]
    else:
        nc_m = _build(cfg, G, meta, per_core[0]["L1"], per_core[0]["L2"])
        _CACHE[bkey] = nc_m

    s1, t1 = _fold_bn(np.asarray(g1), np.asarray(be1), np.asarray(m1),
                      np.asarray(v1), np.asarray(b1))
    s2, t2 = _fold_bn(np.asarray(g2), np.asarray(be2), np.asarray(m2),
                      np.asarray(v2), np.asarray(b2))
    Tg_max = max(max(per_core[0]["L1"]["oh_tiles"]),
                 max(per_core[0]["L2"]["oh_tiles"]))
    iota = np.tile(np.arange(128, dtype=np.float16), (128, Tg_max))
    bf_rep = np.broadcast_to(np.asarray(bf, dtype=np.float32), (128, 2)).copy()

    ncores = cfg["NC"]
    in_maps = []
    for c in range(ncores):
        pc = per_core[c]
        in_maps.append(dict(
            xT=pc["xT"], W1=np.asarray(W1, np.float16),
            W2=np.asarray(W2, np.float16), Wf=np.asarray(Wf, np.float16),
            bf_rep=bf_rep, s1=s1, t1=t1, s2=s2, t2=t2, iota=iota,
            idx1=pc["L1"]["idx16"], idx2=pc["L2"]["idx16"],
            dloc1=pc["L1"]["dloc"], dloc2=pc["L2"]["dloc"],
            ddrow=pc["ddrow"], dinv_nat=pc["dinv_nat"],
            dinv_padlay=pc["dinv_padlay"],
        ))
    res = bass_utils.run_bass_kernel_spmd(
        nc_m, in_maps, core_ids=list(range(ncores)), trace=trace)

    class _Res:
        pass

    r = _Res()
    r.results = res.results
    r.exec_time_ns = res.exec_time_ns
    r.parts = (res,)

    gi = np.asarray(game_indices, dtype=np.int64)
    cji = meta["pad_cji"][gi]
    lp = np.stack([res.results[c]["logp"] for c in range(ncores)])
    out = np.empty((gi.shape[0], 2), dtype=np.float32)
    out[:, 0] = lp[cji[:, 0], cji[:, 2], 2 * cji[:, 1]]
    out[:, 1] = lp[cji[:, 0], cji[:, 2], 2 * cji[:, 1] + 1]
    return out, r


def kernel(**inputs):
    out, _ = _run(**inputs)
    return out


def kernel_profiled(**inputs):
    out, res = _run(**inputs, trace=True)
    return out, res



# revision 10
# speedup vs baseline: 1.4327x; 1.0711x over previous
"""Trainium2 Bass kernel for a 2-layer GCN (EnhancedHockeyGNN) — v5.

v5 = v4 + natural-tile grouping + cohort-packed gather rows:
  - Groups are the natural 128-node tiles of each core's shard (no bin
    packing). Self-loop edges are dropped from the gather entirely — the
    self contribution is one identity matmul from the locally-resident
    xs tile (xsb / xs2b) into the group's PSUM accumulation.
  - Gather rows are packed contiguously per (cohort, chunk) with shared
    boundary tiles (per-group capacity = cross-core max edge count, no
    per-group 128-rounding): ~14% fewer SWDGE descriptors, which is the
    hard bottleneck (all dynamic-DMA rings share one DMA engine).
  - Layer-2 message tables have the same row layout as layer-1's (both
    are in natural node order), so ONE idx/dloc table set serves both
    layers — no mid-kernel table swap.
  - Gather sub-calls are assigned round-robin across the 4 SWDGE queues
    (not queue=chunk), so all queues start as soon as the first
    AllGather chunks land and stay balanced.
"""
import math

import numpy as np

# ---------------------------------------------------------------- constants
N = 100000
F_IN = 128
H = 128
NC = 8
SHARD = 12544            # multiple of 128; 8 * 12544 = 100352 >= N
NPAD = NC * SHARD
NCHUNK = 4               # AllGather chunks == src buckets (int16 idx limit)
COH = 4                  # groups per gather cohort
NQ = 4                   # SWDGE queues
EPS = 1e-5

_CACHE = {}


def _chunks(n, k):
    k = min(k, n)
    base, rem = n // k, n % k
    out, lo = [], 0
    for i in range(k):
        hi = lo + base + (1 if i < rem else 0)
        out.append((lo, hi))
        lo = hi
    return out


def _wrap_idx16(idx_flat):
    """[n] int16 -> [128, n//16] wrapped (i -> [i%16, i//16]) + replicated."""
    n = idx_flat.shape[0]
    assert n % 16 == 0
    w = idx_flat.reshape(n // 16, 16).T            # [16, cols]
    return np.tile(w, (8, 1)).copy()               # [128, cols]


# ---------------------------------------------------------------- host prep
def _prepare(x, edge_index, cfg):
    n, npad, shard, nc = cfg["N"], cfg["NPAD"], cfg["SHARD"], cfg["NC"]
    nchunk, coh, nq = cfg["NCHUNK"], cfg["COH"], cfg["NQ"]
    cap_tiles = cfg.get("NIDX_CAP", 1024) // 128

    src = np.asarray(edge_index[0], dtype=np.int64)
    dst = np.asarray(edge_index[1], dtype=np.int64)
    deg = np.bincount(dst, minlength=n).astype(np.float64) + 1.0
    dinv = (1.0 / np.sqrt(deg)).astype(np.float32)
    dinv_pad_full = np.ones(npad, dtype=np.float32)
    dinv_pad_full[:n] = dinv

    G = shard // 128                       # natural groups per core
    ncoh = (G + coh - 1) // coh
    ch_a = _chunks(G, nchunk)              # chunk q holds tiles [lo, hi)
    rows_a = [nc * (hi - lo) * 128 for lo, hi in ch_a]
    assert max(rows_a) <= 32767 + 1

    tile_q = np.zeros(G, dtype=np.int64)
    for q, (lo, hi) in enumerate(ch_a):
        tile_q[lo:hi] = q
    nodes = np.arange(npad, dtype=np.int64)
    c_of = nodes // shard
    loc = nodes % shard
    j_of = loc // 128
    p_of = loc % 128
    q_of = tile_q[j_of]                    # chunk of node (both layers)
    lo_a = np.array([lo for lo, hi in ch_a], dtype=np.int64)[q_of]
    nt_a = np.array([hi - lo for lo, hi in ch_a], dtype=np.int64)[q_of]
    row_in_chunk = c_of * nt_a * 128 + (j_of - lo_a) * 128 + p_of

    owner = dst // shard
    # per-core edge lists sorted by (group, chunk); self loops excluded
    core_el = []
    e_cgq = np.zeros((nc, G, nchunk), dtype=np.int64)
    for c in range(nc):
        m = owner == c
        s_, d_ = src[m], dst[m]
        g_ = (d_ - c * shard) // 128
        qe = q_of[s_]
        key = g_ * nchunk + qe
        order = np.argsort(key, kind="stable")
        ko = key[order]
        rows_o = row_in_chunk[s_][order]
        dpos_o = (d_[order] % 128).astype(np.int64)
        starts = np.searchsorted(ko, np.arange(G * nchunk))
        ends = np.searchsorted(ko, np.arange(G * nchunk) + 1)
        el = {}
        for g in range(G):
            for q in range(nchunk):
                b = g * nchunk + q
                a_, b_ = int(starts[b]), int(ends[b])
                el[(g, q)] = (rows_o[a_:b_], dpos_o[a_:b_])
                e_cgq[c, g, q] = b_ - a_
        core_el.append(el)

    C_gq = e_cgq.max(axis=0)               # common per-(g,q) capacity

    # ---- common layout: per (cohort, q) contiguous rows, shared boundaries
    # call_meta[h] = list of (queue_num, col_lo, ncols, ntiles, buf_off)
    # grp_meta[g] = list of (buf_off_q + ta, ntiles_g_q) segments
    call_meta = []
    grp_meta = [[] for _ in range(G)]
    oh_tiles = [0] * G
    col_base = 0
    rr = 0
    for h in range(ncoh):
        gs = list(range(h * coh, min((h + 1) * coh, G)))
        meta_h = []
        buf_off = 0
        for q in range(nchunk):
            R = int(C_gq[gs, q].sum())
            T = (R + 127) // 128
            if T == 0:
                continue
            off = 0
            for g in gs:
                cgq = int(C_gq[g, q])
                if cgq > 0:
                    ta, tb = off // 128, (off + cgq - 1) // 128
                    grp_meta[g].append((q, off, cgq, buf_off + ta,
                                        tb - ta + 1))
                    oh_tiles[g] += tb - ta + 1
                off += cgq
            t0 = 0
            while t0 < T:
                tp = min(cap_tiles, T - t0)
                meta_h.append((rr % nq, col_base + t0 * 8, tp * 8, tp,
                               buf_off + t0))
                rr += 1
                t0 += tp
            col_base += T * 8
            buf_off += T
        call_meta.append(meta_h)
    total_tiles = col_base // 8
    n_oh = sum(oh_tiles)

    per_core = []
    for c in range(nc):
        el = core_el[c]
        idx16 = np.zeros(total_tiles * 128, dtype=np.int16)
        dloc = np.full((n_oh, 128), 300.0, dtype=np.float16)
        ohc = 0
        tile_base = 0
        for h in range(ncoh):
            gs = list(range(h * coh, min((h + 1) * coh, G)))
            for q in range(nchunk):
                R = int(C_gq[gs, q].sum())
                T = (R + 127) // 128
                if T == 0:
                    continue
                seg = np.zeros(T * 128, dtype=np.int16)
                segd = np.full(T * 128, 300.0, dtype=np.float16)
                off = 0
                for g in gs:
                    rows_e, dpos_e = el[(g, q)]
                    ne = rows_e.shape[0]
                    seg[off:off + ne] = rows_e.astype(np.int16)
                    segd[off:off + ne] = dpos_e
                    off += int(C_gq[g, q])
                idx16[tile_base * 128:(tile_base + T) * 128] = seg
                tile_base += T
        # dloc columns per group: full 128-spans of its tiles, masked to own
        for g in range(G):
            for (q, off, cgq, mt0, ntl) in grp_meta[g]:
                rows_e, dpos_e = el[(g, q)]
                ne = rows_e.shape[0]
                ta = off // 128
                dl = np.full(ntl * 128, 300.0, dtype=np.float16)
                s0 = off - ta * 128
                dl[s0:s0 + ne] = dpos_e
                dloc[ohc:ohc + ntl] = dl.reshape(ntl, 128)
                ohc += ntl
        assert ohc == n_oh
        idx_w = _wrap_idx16(idx16)                     # [128, total_tiles*8]
        dloc_t = np.ascontiguousarray(dloc.T)          # [128, n_oh]

        jj = np.arange(shard)
        dinv_nat = dinv_pad_full[c * shard + jj].reshape(G, 128).T.copy()
        ddrow = np.broadcast_to(
            dinv_pad_full[c * shard + jj].astype(np.float16)[None, :],
            (128, shard)).copy()
        xs_shape = np.zeros((shard, x.shape[1]), dtype=np.float32)
        lo, hi = c * shard, min((c + 1) * shard, n)
        xs_shape[: hi - lo] = x[lo:hi]
        xT = np.ascontiguousarray(xs_shape.T).astype(np.float16)
        per_core.append(dict(idx16=idx_w, dloc=dloc_t, dinv_nat=dinv_nat,
                             ddrow=ddrow, xT=xT))

    meta = dict(ch_a=ch_a, rows_a=rows_a, call_meta=call_meta,
                grp_meta=grp_meta, oh_tiles=oh_tiles,
                total_tiles=total_tiles, n_oh=n_oh)
    return per_core, meta, G


def _fold_bn(gamma, beta, mean, var, b):
    s = (gamma / np.sqrt(var + EPS)).astype(np.float32)
    t = ((b - mean) * s + beta).astype(np.float32)
    return s.reshape(H, 1), t.reshape(H, 1)


# ---------------------------------------------------------------- bass build
def _build(cfg, G, meta):
    import concourse.bacc as bacc
    import concourse.bass as bass
    import concourse.mybir as mybir
    import concourse.tile as tile

    fp32 = mybir.dt.float32
    fp16 = mybir.dt.float16
    i16 = mybir.dt.int16
    AF = mybir.ActivationFunctionType

    nc_ = cfg["NC"]
    shard = cfg["SHARD"]
    h = cfg["H"]
    fin = cfg["F_IN"]
    nchunk = cfg["NCHUNK"]
    coh = cfg["COH"]
    ch_a = meta["ch_a"]
    rows_a = meta["rows_a"]
    call_meta = meta["call_meta"]
    grp_meta = meta["grp_meta"]
    oh_tiles = meta["oh_tiles"]
    total_tiles = meta["total_tiles"]
    n_oh = meta["n_oh"]
    ncoh = len(call_meta)
    Tg_max = max(oh_tiles)

    nc = bacc.Bacc(None, target_bir_lowering=False, debug=False,
                   num_devices=nc_, num_swdge_queues=cfg["NQ"])

    iota_in = nc.dram_tensor("iota", [128, Tg_max * 128], fp16,
                             kind="ExternalInput")
    dloc_in = nc.dram_tensor("dloc1", [128, n_oh], fp16,
                             kind="ExternalInput")
    idx_in = nc.dram_tensor("idx1", [128, total_tiles * 8], i16,
                            kind="ExternalInput")
    iden_in = nc.dram_tensor("iden", [128, 128], fp16, kind="ExternalInput")
    ddrow_in = nc.dram_tensor("ddrow", [128, shard], fp16,
                              kind="ExternalInput")
    xT_in = nc.dram_tensor("xT", [fin, shard], fp16, kind="ExternalInput")
    w1_in = nc.dram_tensor("W1", [fin, h], fp16, kind="ExternalInput")
    w2_in = nc.dram_tensor("W2", [h, h], fp16, kind="ExternalInput")
    wf_in = nc.dram_tensor("Wf", [h, 2], fp16, kind="ExternalInput")
    bf_in = nc.dram_tensor("bf_rep", [128, 2], fp32, kind="ExternalInput")
    s1_in = nc.dram_tensor("s1", [h, 1], fp32, kind="ExternalInput")
    t1_in = nc.dram_tensor("t1", [h, 1], fp32, kind="ExternalInput")
    s2_in = nc.dram_tensor("s2", [h, 1], fp32, kind="ExternalInput")
    t2_in = nc.dram_tensor("t2", [h, 1], fp32, kind="ExternalInput")
    dn_in = nc.dram_tensor("dinv_nat", [128, G], fp32, kind="ExternalInput")
    out_lp = nc.dram_tensor("logp", [128, 2 * G], fp32, kind="ExternalOutput")

    with tile.TileContext(nc) as tc:
        with (
            tc.tile_pool(name="res", bufs=1) as res,
            tc.tile_pool(name="big", bufs=1) as big,
            tc.tile_pool(name="stream", bufs=1) as st,
            tc.tile_pool(name="ps", bufs=1, space="PSUM") as ps,
            tc.tile_pool(name="dram", bufs=1, space="DRAM") as dram,
        ):
            iota_t = res.tile([128, Tg_max, 128], fp16)
            dloc_t = res.tile([128, n_oh], fp16)
            idx_t = res.tile([128, total_tiles * 8], i16)
            iden_t = res.tile([128, 128], fp16)
            ddrow_t = res.tile([128, shard], fp16)
            w1_t = res.tile([fin, h], fp16)
            w2_t = res.tile([h, h], fp16)
            wf_t = res.tile([h, 2], fp16)
            bf_t = res.tile([128, 2], fp32)
            s1_t = res.tile([h, 1], fp32)
            t1_t = res.tile([h, 1], fp32)
            s2_t = res.tile([h, 1], fp32)
            t2_t = res.tile([h, 1], fp32)
            dn_t = res.tile([128, G], fp32)
            nc.sync.dma_start(out=iota_t[:],
                              in_=iota_in[:].rearrange("p (k d) -> p k d",
                                                       d=128))
            nc.sync.dma_start(out=ddrow_t[:], in_=ddrow_in[:])
            for t_, i_ in ((iden_t, iden_in), (w1_t, w1_in), (w2_t, w2_in),
                           (wf_t, wf_in), (bf_t, bf_in), (s1_t, s1_in),
                           (t1_t, t1_in), (s2_t, s2_in), (t2_t, t2_in),
                           (dn_t, dn_in)):
                nc.sync.dma_start(out=t_[:], in_=i_[:])
            nc.sync.dma_start(out=idx_t[:], in_=idx_in[:])
            nc.sync.dma_start(out=dloc_t[:], in_=dloc_in[:])

            # ohcol[g]: first dloc/one-hot column of group g
            ohcol = [0] * G
            acc = 0
            for g in range(G):
                ohcol[g] = acc
                acc += oh_tiles[g]

            def edge_layer(tables, xself, s_t, t_t, post_group):
                for hcoh in range(ncoh):
                    gs = list(range(hcoh * coh, min((hcoh + 1) * coh, G)))
                    T_h = sum(m[3] for m in call_meta[hcoh])
                    msg = st.tile([128, T_h, h], fp16, name="msg", tag="msg",
                                  bufs=2)
                    for (qn, col_lo, ncols, ntq, off) in call_meta[hcoh]:
                        nidx = ntq * 128
                        nc.gpsimd.dma_gather(
                            msg[:, off:off + ntq, :],
                            tables[q_of_call(col_lo)],
                            idx_t[:, col_lo:col_lo + ncols],
                            nidx,
                            nidx,
                            h,
                            queue_num=qn,
                        )
                    for g in gs:
                        Tg = oh_tiles[g]
                        oh = st.tile([128, Tg_max, 128], fp16, name="oh",
                                     tag="oh", bufs=3)
                        if Tg > 0:
                            nc.vector.tensor_tensor(
                                out=oh[:, :Tg, :],
                                in0=iota_t[:, :Tg, :],
                                in1=dloc_t[:, ohcol[g]:ohcol[g] + Tg]
                                    .to_broadcast([128, Tg, 128]),
                                op=mybir.AluOpType.is_equal,
                            )
                        pg = ps.tile([h, 128], fp32, name="pg", tag="pg",
                                     bufs=4)
                        # self-loop term: xs[tile g]^T via identity
                        nc.tensor.matmul(pg[:],
                                         xself[:, g * 128:(g + 1) * 128],
                                         iden_t[:],
                                         start=True, stop=(Tg == 0))
                        i = 0
                        for (q, off, cgq, mt0, ntl) in grp_meta[g]:
                            for t in range(ntl):
                                nc.tensor.matmul(pg[:], msg[:, mt0 + t, :],
                                                 oh[:, i, :],
                                                 start=False,
                                                 stop=(i == Tg - 1))
                                i += 1
                        tmp = st.tile([h, 128], fp32, name="tmp", tag="tmp",
                                      bufs=4)
                        nc.vector.tensor_tensor(
                            out=tmp[:], in0=pg[:],
                            in1=ddrow_t[:, g * 128:(g + 1) * 128],
                            op=mybir.AluOpType.mult,
                        )
                        hblk = st.tile([h, 128], fp16, name="hblk",
                                       tag="hblk", bufs=4)
                        nc.scalar.activation(
                            out=hblk[:], in_=tmp[:],
                            func=AF.Relu, bias=t_t[:], scale=s_t[:],
                        )
                        post_group(g, hblk)

            _colq = meta["colq"]

            def q_of_call(col_lo):
                return _colq[col_lo]

            # ---- stage A: xs1 compute, staged + AllGather'd per chunk
            xs1_shard = dram.tile([shard, h], fp16)
            xs1_q = [dram.tile([rows_a[q], h], fp16, addr_space="Shared",
                               name=f"xs1q{q}")
                     for q in range(nchunk)]
            xsb = big.tile([128, G * 128], fp16, name="xsb", tag="big_a")
            for q, (lo, hi) in enumerate(ch_a):
                for j in range(lo, hi):
                    lhsT = st.tile([128, 128], fp16, name="xTt",
                                   tag="lhsT", bufs=4)
                    nc.sync.dma_start(
                        out=lhsT[:], in_=xT_in[:, j * 128:(j + 1) * 128])
                    pxs = ps.tile([128, h], fp32, name="pxs", tag="pxs",
                                  bufs=2)
                    nc.tensor.matmul(pxs[:], lhsT[:], w1_t[:], start=True,
                                     stop=True)
                    nc.vector.tensor_scalar(
                        out=xsb[:, j * 128:(j + 1) * 128], in0=pxs[:],
                        scalar1=dn_t[:, j:j + 1], scalar2=None,
                        op0=mybir.AluOpType.mult)
                rows = hi - lo
                dest = bass.AP(xs1_shard[:].tensor, lo * 128 * h,
                               [[h, 128], [128 * h, rows], [1, h]])
                nc.sync.dma_start(out=dest, in_=xsb[:].rearrange(
                    "p (j f) -> p j f", f=h)[:, lo:hi, :])
                nc.gpsimd.collective_compute(
                    "AllGather", mybir.AluOpType.bypass,
                    replica_groups=[list(range(nc_))],
                    ins=[xs1_shard[lo * 128:hi * 128, :].opt()],
                    outs=[xs1_q[q][:].opt()],
                )

            # ---- layer 1 with interleaved xs2 production + AG2
            xs2_shard = dram.tile([G * 128, h], fp16)
            xs2q_int = [dram.tile([rows_a[q], h], fp16,
                                  addr_space="Shared", name=f"xs2qi{q}")
                        for q in range(nchunk)]
            xs2b = big.tile([128, G * 128], fp16, name="xs2b", tag="big_c")
            g_last = {hi - 1: q for q, (lo, hi) in enumerate(ch_a)}

            def post_group_a(g, hblk):
                pxs = ps.tile([128, h], fp32, name="pxs2", tag="pxs",
                              bufs=2)
                nc.tensor.matmul(pxs[:], hblk[:], w2_t[:], start=True,
                                 stop=True)
                nc.vector.tensor_scalar(
                    out=xs2b[:, g * 128:(g + 1) * 128], in0=pxs[:],
                    scalar1=dn_t[:, g:g + 1], scalar2=None,
                    op0=mybir.AluOpType.mult)
                if g in g_last:
                    q = g_last[g]
                    lo, hi = ch_a[q]
                    rows = hi - lo
                    dest = bass.AP(xs2_shard[:].tensor, lo * 128 * h,
                                   [[h, 128], [128 * h, rows], [1, h]])
                    nc.sync.dma_start(out=dest, in_=xs2b[:].rearrange(
                        "p (j f) -> p j f", f=h)[:, lo:hi, :])
                    nc.gpsimd.collective_compute(
                        "AllGather", mybir.AluOpType.bypass,
                        replica_groups=[list(range(nc_))],
                        ins=[xs2_shard[lo * 128:hi * 128, :].opt()],
                        outs=[xs2q_int[q][:].opt()],
                    )

            edge_layer([t[:] for t in xs1_q], xsb[:], s1_t, t1_t,
                       post_group_a)

            lg = res.tile([128, 2 * G], fp32)

            def post_group_b(g, hblk):
                plg = ps.tile([128, 2], fp32, name="plg", tag="plg",
                              bufs=2)
                nc.tensor.matmul(plg[:], hblk[:], wf_t[:], start=True,
                                 stop=True)
                nc.vector.tensor_add(out=lg[:, 2 * g:2 * g + 2],
                                     in0=plg[:], in1=bf_t[:])

            edge_layer([t[:] for t in xs2q_int], xs2b[:], s2_t, t2_t,
                       post_group_b)

            def strided(base, start):
                a = base[:]
                return bass.AP(a.tensor, a.offset + start,
                               [a.ap[0], [2, G]])

            z0, z1 = strided(lg, 0), strided(lg, 1)
            mx = res.tile([128, G], fp32)
            nc.vector.tensor_tensor(out=mx[:], in0=z0, in1=z1,
                                    op=mybir.AluOpType.max)
            sm0 = res.tile([128, G], fp32)
            sm1 = res.tile([128, G], fp32)
            nc.vector.tensor_sub(out=sm0[:], in0=z0, in1=mx[:])
            nc.vector.tensor_sub(out=sm1[:], in0=z1, in1=mx[:])
            e0 = res.tile([128, G], fp32)
            e1 = res.tile([128, G], fp32)
            nc.scalar.activation(out=e0[:], in_=sm0[:], func=AF.Exp)
            nc.scalar.activation(out=e1[:], in_=sm1[:], func=AF.Exp)
            se = res.tile([128, G], fp32)
            nc.vector.tensor_add(out=se[:], in0=e0[:], in1=e1[:])
            ls = res.tile([128, G], fp32)
            nc.scalar.activation(out=ls[:], in_=se[:], func=AF.Ln)
            nc.vector.tensor_sub(out=sm0[:], in0=sm0[:], in1=ls[:])
            nc.vector.tensor_sub(out=sm1[:], in0=sm1[:], in1=ls[:])
            lpo = res.tile([128, 2 * G], fp32)
            nc.vector.tensor_copy(out=strided(lpo, 0), in_=sm0[:])
            nc.vector.tensor_copy(out=strided(lpo, 1), in_=sm1[:])
            nc.sync.dma_start(out=out_lp[:], in_=lpo[:])

    nc.compile()
    return nc


# ---------------------------------------------------------------- main entry
def _run(x, edge_index, game_indices,
         W1, b1, g1, be1, m1, v1, W2, b2, g2, be2, m2, v2, Wf, bf,
         trace=False, cfg=None):
    from concourse import bass_utils

    if cfg is None:
        cfg = dict(N=N, NPAD=NPAD, SHARD=SHARD, NC=NC, H=H, F_IN=F_IN,
                   NCHUNK=NCHUNK, COH=COH, NQ=NQ, NIDX_CAP=1024)

    x = np.asarray(x, dtype=np.float32)
    key = ("prep", x.shape, int(np.asarray(edge_index)[0, 0]),
           int(np.asarray(edge_index).sum() % (1 << 31)))
    if key in _CACHE:
        per_core, meta, G = _CACHE[key]
    else:
        per_core, meta, G = _prepare(x, np.asarray(edge_index), cfg)
        # colq: map call col_lo -> chunk q (for table selection)
        colq = {}
        ncoh = len(meta["call_meta"])
        coh = cfg["COH"]
        col = 0
        # reproduce prep's chunk-major col layout
        import numpy as _np
        C_gq = _np.zeros((G, cfg["NCHUNK"]), dtype=_np.int64)
        for g in range(G):
            for (q, off, cgq, mt0, ntl) in meta["grp_meta"][g]:
                C_gq[g, q] += cgq
        for h in range(ncoh):
            gs = list(range(h * coh, min((h + 1) * coh, G)))
            for q in range(cfg["NCHUNK"]):
                R = int(C_gq[gs, q].sum())
                T = (R + 127) // 128
                for t0 in range(0, T, cfg["NIDX_CAP"] // 128):
                    colq[col + t0 * 8] = q
                col += T * 8
        meta["colq"] = colq
        _CACHE.clear()
        _CACHE[key] = (per_core, meta, G)

    bkey = ("bass", G, meta["total_tiles"], meta["n_oh"],
            tuple(tuple(m) for h_ in meta["call_meta"] for m in h_))
    if bkey in _CACHE:
        nc_m = _CACHE[bkey]
    else:
        nc_m = _build(cfg, G, meta)
        _CACHE[bkey] = nc_m

    s1, t1 = _fold_bn(np.asarray(g1), np.asarray(be1), np.asarray(m1),
                      np.asarray(v1), np.asarray(b1))
    s2, t2 = _fold_bn(np.asarray(g2), np.asarray(be2), np.asarray(m2),
                      np.asarray(v2), np.asarray(b2))
    Tg_max = max(meta["oh_tiles"])
    iota = np.tile(np.arange(128, dtype=np.float16), (128, Tg_max))
    iden = np.eye(128, dtype=np.float16)
    bf_rep = np.broadcast_to(np.asarray(bf, dtype=np.float32), (128, 2)).copy()

    ncores = cfg["NC"]
    in_maps = []
    for c in range(ncores):
        pc = per_core[c]
        in_maps.append(dict(
            xT=pc["xT"], W1=np.asarray(W1, np.float16),
            W2=np.asarray(W2, np.float16), Wf=np.asarray(Wf, np.float16),
            bf_rep=bf_rep, s1=s1, t1=t1, s2=s2, t2=t2, iota=iota,
            iden=iden, idx1=pc["idx16"], dloc1=pc["dloc"],
            ddrow=pc["ddrow"], dinv_nat=pc["dinv_nat"],
        ))
    res = bass_utils.run_bass_kernel_spmd(
        nc_m, in_maps, core_ids=list(range(ncores)), trace=trace)

    class _Res:
        pass

    r = _Res()
    r.results = res.results
    r.exec_time_ns = res.exec_time_ns
    r.parts = (res,)

    gi = np.asarray(game_indices, dtype=np.int64)
    shard = cfg["SHARD"]
    ci = gi // shard
    gidx = (gi % shard) // 128
    pi = gi % 128
    lp = np.stack([res.results[c]["logp"] for c in range(ncores)])
    out = np.empty((gi.shape[0], 2), dtype=np.float32)
    out[:, 0] = lp[ci, pi, 2 * gidx]
    out[:, 1] = lp[ci, pi, 2 * gidx + 1]
    return out, r


def kernel(**inputs):
    out, _ = _run(**inputs)
    return out


def kernel_profiled(**inputs):
    out, res = _run(**inputs, trace=True)
    return out, res


# revision 11
# speedup vs baseline: 2.0630x; 1.4399x over previous
"""Trainium2 Bass kernel for a 2-layer GCN (EnhancedHockeyGNN) — v5.

v5 = v4 + natural-tile grouping + cohort-packed gather rows:
  - Groups are the natural 128-node tiles of each core's shard (no bin
    packing). Self-loop edges are dropped from the gather entirely — the
    self contribution is one identity matmul from the locally-resident
    xs tile (xsb / xs2b) into the group's PSUM accumulation.
  - Gather rows are packed contiguously per (cohort, chunk) with shared
    boundary tiles (per-group capacity = cross-core max edge count, no
    per-group 128-rounding): ~14% fewer SWDGE descriptors, which is the
    hard bottleneck (all dynamic-DMA rings share one DMA engine).
  - Layer-2 message tables have the same row layout as layer-1's (both
    are in natural node order), so ONE idx/dloc table set serves both
    layers — no mid-kernel table swap.
  - Gather sub-calls are assigned round-robin across the 4 SWDGE queues
    (not queue=chunk), so all queues start as soon as the first
    AllGather chunks land and stay balanced.
"""
import math

import numpy as np

# ---------------------------------------------------------------- constants
N = 100000
F_IN = 128
H = 128
NC = 8
SHARD = 12544            # multiple of 128; 8 * 12544 = 100352 >= N
NPAD = NC * SHARD
NCHUNK = 4               # AllGather chunks == src buckets (int16 idx limit)
COH = 4                  # groups per gather cohort
NQ = 4                   # SWDGE queues
EPS = 1e-5

_CACHE = {}


def _chunks(n, k):
    k = min(k, n)
    base, rem = n // k, n % k
    out, lo = [], 0
    for i in range(k):
        hi = lo + base + (1 if i < rem else 0)
        out.append((lo, hi))
        lo = hi
    return out


def _wrap_idx16(idx_flat):
    """[n] int16 -> [128, n//16] wrapped (i -> [i%16, i//16]) + replicated."""
    n = idx_flat.shape[0]
    assert n % 16 == 0
    w = idx_flat.reshape(n // 16, 16).T            # [16, cols]
    return np.tile(w, (8, 1)).copy()               # [128, cols]


# ---------------------------------------------------------------- host prep
def _prepare(x, edge_index, cfg):
    n, npad, shard, nc = cfg["N"], cfg["NPAD"], cfg["SHARD"], cfg["NC"]
    nchunk, coh, nq = cfg["NCHUNK"], cfg["COH"], cfg["NQ"]
    cap_tiles = cfg.get("NIDX_CAP", 1024) // 128

    src = np.asarray(edge_index[0], dtype=np.int64)
    dst = np.asarray(edge_index[1], dtype=np.int64)
    deg = np.bincount(dst, minlength=n).astype(np.float64) + 1.0
    dinv = (1.0 / np.sqrt(deg)).astype(np.float32)
    dinv_pad_full = np.ones(npad, dtype=np.float32)
    dinv_pad_full[:n] = dinv

    G = shard // 128                       # natural groups per core
    ncoh = (G + coh - 1) // coh
    # uneven chunks: small chunk 0 so AllGather-0 (and the first gathers)
    # start early; max 30 tiles keeps chunk tables < 32768 rows (int16)
    sizes = [10, 30, 29, 29]
    assert sum(sizes) == G and len(sizes) == nchunk
    ch_a = []
    lo = 0
    for s_ in sizes:
        ch_a.append((lo, lo + s_))
        lo += s_
    rows_a = [nc * (hi - lo) * 128 for lo, hi in ch_a]
    assert max(rows_a) <= 32767

    tile_q = np.zeros(G, dtype=np.int64)
    for q, (lo, hi) in enumerate(ch_a):
        tile_q[lo:hi] = q
    nodes = np.arange(npad, dtype=np.int64)
    c_of = nodes // shard
    loc = nodes % shard
    j_of = loc // 128
    p_of = loc % 128
    q_of = tile_q[j_of]                    # chunk of node (both layers)
    lo_a = np.array([lo for lo, hi in ch_a], dtype=np.int64)[q_of]
    nt_a = np.array([hi - lo for lo, hi in ch_a], dtype=np.int64)[q_of]
    row_in_chunk = c_of * nt_a * 128 + (j_of - lo_a) * 128 + p_of

    owner = dst // shard
    # per-core edge lists sorted by (group, chunk); self loops excluded
    core_el = []
    e_cgq = np.zeros((nc, G, nchunk), dtype=np.int64)
    for c in range(nc):
        m = owner == c
        s_, d_ = src[m], dst[m]
        g_ = (d_ - c * shard) // 128
        qe = q_of[s_]
        key = g_ * nchunk + qe
        order = np.argsort(key, kind="stable")
        ko = key[order]
        rows_o = row_in_chunk[s_][order]
        dpos_o = (d_[order] % 128).astype(np.int64)
        starts = np.searchsorted(ko, np.arange(G * nchunk))
        ends = np.searchsorted(ko, np.arange(G * nchunk) + 1)
        el = {}
        for g in range(G):
            for q in range(nchunk):
                b = g * nchunk + q
                a_, b_ = int(starts[b]), int(ends[b])
                el[(g, q)] = (rows_o[a_:b_], dpos_o[a_:b_])
                e_cgq[c, g, q] = b_ - a_
        core_el.append(el)

    C_gq = e_cgq.max(axis=0)               # common per-(g,q) capacity

    # ---- common layout: per (cohort, q) contiguous rows, shared boundaries
    # call_meta[h] = list of (queue_num, col_lo, ncols, ntiles, buf_off)
    # grp_meta[g] = list of (buf_off_q + ta, ntiles_g_q) segments
    call_meta = []
    grp_meta = [[] for _ in range(G)]
    oh_tiles = [0] * G
    col_base = 0
    rr = 0
    for h in range(ncoh):
        gs = list(range(h * coh, min((h + 1) * coh, G)))
        meta_h = []
        buf_off = 0
        for q in range(nchunk):
            R = int(C_gq[gs, q].sum())
            T = (R + 127) // 128
            if T == 0:
                continue
            off = 0
            for g in gs:
                cgq = int(C_gq[g, q])
                if cgq > 0:
                    ta, tb = off // 128, (off + cgq - 1) // 128
                    grp_meta[g].append((q, off, cgq, buf_off + ta,
                                        tb - ta + 1))
                    oh_tiles[g] += tb - ta + 1
                off += cgq
            t0 = 0
            while t0 < T:
                tp = min(cap_tiles, T - t0)
                meta_h.append((rr % nq, col_base + t0 * 8, tp * 8, tp,
                               buf_off + t0))
                rr += 1
                t0 += tp
            col_base += T * 8
            buf_off += T
        call_meta.append(meta_h)
    total_tiles = col_base // 8
    n_oh = sum(oh_tiles)

    per_core = []
    for c in range(nc):
        el = core_el[c]
        idx16 = np.zeros(total_tiles * 128, dtype=np.int16)
        dloc = np.full((n_oh, 128), 300.0, dtype=np.float16)
        ohc = 0
        tile_base = 0
        for h in range(ncoh):
            gs = list(range(h * coh, min((h + 1) * coh, G)))
            for q in range(nchunk):
                R = int(C_gq[gs, q].sum())
                T = (R + 127) // 128
                if T == 0:
                    continue
                seg = np.zeros(T * 128, dtype=np.int16)
                segd = np.full(T * 128, 300.0, dtype=np.float16)
                off = 0
                for g in gs:
                    rows_e, dpos_e = el[(g, q)]
                    ne = rows_e.shape[0]
                    seg[off:off + ne] = rows_e.astype(np.int16)
                    segd[off:off + ne] = dpos_e
                    off += int(C_gq[g, q])
                idx16[tile_base * 128:(tile_base + T) * 128] = seg
                tile_base += T
        # dloc columns per group: full 128-spans of its tiles, masked to own
        for g in range(G):
            for (q, off, cgq, mt0, ntl) in grp_meta[g]:
                rows_e, dpos_e = el[(g, q)]
                ne = rows_e.shape[0]
                ta = off // 128
                dl = np.full(ntl * 128, 300.0, dtype=np.float16)
                s0 = off - ta * 128
                dl[s0:s0 + ne] = dpos_e
                dloc[ohc:ohc + ntl] = dl.reshape(ntl, 128)
                ohc += ntl
        assert ohc == n_oh
        idx_w = _wrap_idx16(idx16)                     # [128, total_tiles*8]
        dloc_t = np.ascontiguousarray(dloc.T)          # [128, n_oh]

        jj = np.arange(shard)
        dinv_nat = dinv_pad_full[c * shard + jj].reshape(G, 128).T.copy()
        ddrow = np.broadcast_to(
            dinv_pad_full[c * shard + jj].astype(np.float16)[None, :],
            (128, shard)).copy()
        xs_shape = np.zeros((shard, x.shape[1]), dtype=np.float32)
        lo, hi = c * shard, min((c + 1) * shard, n)
        xs_shape[: hi - lo] = x[lo:hi]
        xT = np.ascontiguousarray(xs_shape.T).astype(np.float16)
        per_core.append(dict(idx16=idx_w, dloc=dloc_t, dinv_nat=dinv_nat,
                             ddrow=ddrow, xT=xT))

    meta = dict(ch_a=ch_a, rows_a=rows_a, call_meta=call_meta,
                grp_meta=grp_meta, oh_tiles=oh_tiles,
                total_tiles=total_tiles, n_oh=n_oh)
    return per_core, meta, G


def _fold_bn(gamma, beta, mean, var, b):
    s = (gamma / np.sqrt(var + EPS)).astype(np.float32)
    t = ((b - mean) * s + beta).astype(np.float32)
    return s.reshape(H, 1), t.reshape(H, 1)


# ---------------------------------------------------------------- bass build
def _build(cfg, G, meta):
    import concourse.bacc as bacc
    import concourse.bass as bass
    import concourse.mybir as mybir
    import concourse.tile as tile

    fp32 = mybir.dt.float32
    fp16 = mybir.dt.float16
    i16 = mybir.dt.int16
    AF = mybir.ActivationFunctionType

    nc_ = cfg["NC"]
    shard = cfg["SHARD"]
    h = cfg["H"]
    fin = cfg["F_IN"]
    nchunk = cfg["NCHUNK"]
    coh = cfg["COH"]
    ch_a = meta["ch_a"]
    rows_a = meta["rows_a"]
    call_meta = meta["call_meta"]
    grp_meta = meta["grp_meta"]
    oh_tiles = meta["oh_tiles"]
    total_tiles = meta["total_tiles"]
    n_oh = meta["n_oh"]
    ncoh = len(call_meta)
    Tg_max = max(oh_tiles)

    nc = bacc.Bacc(None, target_bir_lowering=False, debug=False,
                   num_devices=nc_, num_swdge_queues=cfg["NQ"])

    iota_in = nc.dram_tensor("iota", [128, Tg_max * 128], fp16,
                             kind="ExternalInput")
    dloc_in = nc.dram_tensor("dloc1", [128, n_oh], fp16,
                             kind="ExternalInput")
    idx_in = nc.dram_tensor("idx1", [128, total_tiles * 8], i16,
                            kind="ExternalInput")
    iden_in = nc.dram_tensor("iden", [128, 128], fp16, kind="ExternalInput")
    ddrow_in = nc.dram_tensor("ddrow", [128, shard], fp16,
                              kind="ExternalInput")
    xT_in = nc.dram_tensor("xT", [fin, shard], fp16, kind="ExternalInput")
    w1_in = nc.dram_tensor("W1", [fin, h], fp16, kind="ExternalInput")
    w2_in = nc.dram_tensor("W2", [h, h], fp16, kind="ExternalInput")
    wf_in = nc.dram_tensor("Wf", [h, 2], fp16, kind="ExternalInput")
    bf_in = nc.dram_tensor("bf_rep", [128, 2], fp32, kind="ExternalInput")
    s1_in = nc.dram_tensor("s1", [h, 1], fp32, kind="ExternalInput")
    t1_in = nc.dram_tensor("t1", [h, 1], fp32, kind="ExternalInput")
    s2_in = nc.dram_tensor("s2", [h, 1], fp32, kind="ExternalInput")
    t2_in = nc.dram_tensor("t2", [h, 1], fp32, kind="ExternalInput")
    dn_in = nc.dram_tensor("dinv_nat", [128, G], fp32, kind="ExternalInput")
    out_lp = nc.dram_tensor("logp", [128, 2 * G], fp32, kind="ExternalOutput")

    with tile.TileContext(nc) as tc:
        with (
            tc.tile_pool(name="res", bufs=1) as res,
            tc.tile_pool(name="big", bufs=1) as big,
            tc.tile_pool(name="stream", bufs=1) as st,
            tc.tile_pool(name="ps", bufs=1, space="PSUM") as ps,
            tc.tile_pool(name="dram", bufs=1, space="DRAM") as dram,
        ):
            iota_t = res.tile([128, Tg_max, 128], fp16)
            dloc_t = res.tile([128, n_oh], fp16)
            idx_t = res.tile([128, total_tiles * 8], i16)
            iden_t = res.tile([128, 128], fp16)
            ddrow_t = res.tile([128, shard], fp16)
            w1_t = res.tile([fin, h], fp16)
            w2_t = res.tile([h, h], fp16)
            wf_t = res.tile([h, 2], fp16)
            bf_t = res.tile([128, 2], fp32)
            s1_t = res.tile([h, 1], fp32)
            t1_t = res.tile([h, 1], fp32)
            s2_t = res.tile([h, 1], fp32)
            t2_t = res.tile([h, 1], fp32)
            dn_t = res.tile([128, G], fp32)
            # ohcol[g]: first dloc/one-hot column of group g
            ohcol = [0] * G
            acc = 0
            for g in range(G):
                ohcol[g] = acc
                acc += oh_tiles[g]

            for t_, i_ in ((iden_t, iden_in), (w1_t, w1_in), (w2_t, w2_in),
                           (wf_t, wf_in), (bf_t, bf_in), (s1_t, s1_in),
                           (t1_t, t1_in), (s2_t, s2_in), (t2_t, t2_in),
                           (dn_t, dn_in)):
                nc.sync.dma_start(out=t_[:], in_=i_[:])
            nc.sync.dma_start(out=iota_t[:],
                              in_=iota_in[:].rearrange("p (k d) -> p k d",
                                                       d=128))

            # progressive idx/dloc loads: piece 0 lands before the first
            # gathers; later pieces stream behind stage A's chunk loop
            cb = [0] * (ncoh + 1)
            ob = [0] * (ncoh + 1)
            for h_ in range(ncoh):
                ce = cb[h_]
                for (qn, col_lo, ncols, ntq, off) in call_meta[h_]:
                    ce = max(ce, col_lo + ncols)
                cb[h_ + 1] = ce
                ge = min((h_ + 1) * coh, G)
                ob[h_ + 1] = ohcol[ge - 1] + oh_tiles[ge - 1]
            pieces = [(0, 1)] + [(a, b) for a, b in
                                 zip([1, 9, 17], [9, 17, ncoh])]

            def load_piece(k):
                hlo, hhi = pieces[k]
                c0, c1 = cb[hlo], cb[hhi]
                o0, o1 = ob[hlo], ob[hhi]
                if c1 > c0:
                    nc.sync.dma_start(out=idx_t[:, c0:c1],
                                      in_=idx_in[:, c0:c1])
                if o1 > o0:
                    nc.sync.dma_start(out=dloc_t[:, o0:o1],
                                      in_=dloc_in[:, o0:o1])

            load_piece(0)
            nc.sync.dma_start(out=ddrow_t[:], in_=ddrow_in[:])

            def edge_layer(tables, xself, s_t, t_t, post_group):
                for hcoh in range(ncoh):
                    gs = list(range(hcoh * coh, min((hcoh + 1) * coh, G)))
                    T_h = sum(m[3] for m in call_meta[hcoh])
                    msg = st.tile([128, T_h, h], fp16, name="msg", tag="msg",
                                  bufs=2)
                    for (qn, col_lo, ncols, ntq, off) in call_meta[hcoh]:
                        nidx = ntq * 128
                        nc.gpsimd.dma_gather(
                            msg[:, off:off + ntq, :],
                            tables[q_of_call(col_lo)],
                            idx_t[:, col_lo:col_lo + ncols],
                            nidx,
                            nidx,
                            h,
                            queue_num=qn,
                        )
                    for g in gs:
                        Tg = oh_tiles[g]
                        oh = st.tile([128, Tg_max, 128], fp16, name="oh",
                                     tag="oh", bufs=3)
                        if Tg > 0:
                            nc.vector.tensor_tensor(
                                out=oh[:, :Tg, :],
                                in0=iota_t[:, :Tg, :],
                                in1=dloc_t[:, ohcol[g]:ohcol[g] + Tg]
                                    .to_broadcast([128, Tg, 128]),
                                op=mybir.AluOpType.is_equal,
                            )
                        pg = ps.tile([h, 128], fp32, name="pg", tag="pg",
                                     bufs=4)
                        # self-loop term: xs[tile g]^T via identity
                        nc.tensor.matmul(pg[:],
                                         xself[:, g * 128:(g + 1) * 128],
                                         iden_t[:],
                                         start=True, stop=(Tg == 0))
                        i = 0
                        for (q, off, cgq, mt0, ntl) in grp_meta[g]:
                            for t in range(ntl):
                                nc.tensor.matmul(pg[:], msg[:, mt0 + t, :],
                                                 oh[:, i, :],
                                                 start=False,
                                                 stop=(i == Tg - 1))
                                i += 1
                        tmp = st.tile([h, 128], fp32, name="tmp", tag="tmp",
                                      bufs=4)
                        nc.vector.tensor_tensor(
                            out=tmp[:], in0=pg[:],
                            in1=ddrow_t[:, g * 128:(g + 1) * 128],
                            op=mybir.AluOpType.mult,
                        )
                        hblk = st.tile([h, 128], fp16, name="hblk",
                                       tag="hblk", bufs=4)
                        nc.scalar.activation(
                            out=hblk[:], in_=tmp[:],
                            func=AF.Relu, bias=t_t[:], scale=s_t[:],
                        )
                        post_group(g, hblk)

            _colq = meta["colq"]

            def q_of_call(col_lo):
                return _colq[col_lo]

            # ---- stage A: xs1 compute, staged + AllGather'd per chunk
            xs1_shard = dram.tile([shard, h], fp16)
            xs1_q = [dram.tile([rows_a[q], h], fp16, addr_space="Shared",
                               name=f"xs1q{q}")
                     for q in range(nchunk)]
            xsb = big.tile([128, G * 128], fp16, name="xsb", tag="big_a")
            for q, (lo, hi) in enumerate(ch_a):
                for j in range(lo, hi):
                    lhsT = st.tile([128, 128], fp16, name="xTt",
                                   tag="lhsT", bufs=4)
                    nc.sync.dma_start(
                        out=lhsT[:], in_=xT_in[:, j * 128:(j + 1) * 128])
                    pxs = ps.tile([128, h], fp32, name="pxs", tag="pxs",
                                  bufs=2)
                    nc.tensor.matmul(pxs[:], lhsT[:], w1_t[:], start=True,
                                     stop=True)
                    nc.vector.tensor_scalar(
                        out=xsb[:, j * 128:(j + 1) * 128], in0=pxs[:],
                        scalar1=dn_t[:, j:j + 1], scalar2=None,
                        op0=mybir.AluOpType.mult)
                rows = hi - lo
                dest = bass.AP(xs1_shard[:].tensor, lo * 128 * h,
                               [[h, 128], [128 * h, rows], [1, h]])
                nc.sync.dma_start(out=dest, in_=xsb[:].rearrange(
                    "p (j f) -> p j f", f=h)[:, lo:hi, :])
                nc.gpsimd.collective_compute(
                    "AllGather", mybir.AluOpType.bypass,
                    replica_groups=[list(range(nc_))],
                    ins=[xs1_shard[lo * 128:hi * 128, :].opt()],
                    outs=[xs1_q[q][:].opt()],
                )
                if q + 1 < len(pieces):
                    load_piece(q + 1)

            # ---- layer 1 with interleaved xs2 production + AG2
            xs2_shard = dram.tile([G * 128, h], fp16)
            xs2q_int = [dram.tile([rows_a[q], h], fp16,
                                  addr_space="Shared", name=f"xs2qi{q}")
                        for q in range(nchunk)]
            xs2b = big.tile([128, G * 128], fp16, name="xs2b", tag="big_c")
            g_last = {hi - 1: q for q, (lo, hi) in enumerate(ch_a)}

            def post_group_a(g, hblk):
                pxs = ps.tile([128, h], fp32, name="pxs2", tag="pxs",
                              bufs=2)
                nc.tensor.matmul(pxs[:], hblk[:], w2_t[:], start=True,
                                 stop=True)
                nc.vector.tensor_scalar(
                    out=xs2b[:, g * 128:(g + 1) * 128], in0=pxs[:],
                    scalar1=dn_t[:, g:g + 1], scalar2=None,
                    op0=mybir.AluOpType.mult)
                if g in g_last:
                    q = g_last[g]
                    lo, hi = ch_a[q]
                    rows = hi - lo
                    dest = bass.AP(xs2_shard[:].tensor, lo * 128 * h,
                                   [[h, 128], [128 * h, rows], [1, h]])
                    nc.sync.dma_start(out=dest, in_=xs2b[:].rearrange(
                        "p (j f) -> p j f", f=h)[:, lo:hi, :])
                    nc.gpsimd.collective_compute(
                        "AllGather", mybir.AluOpType.bypass,
                        replica_groups=[list(range(nc_))],
                        ins=[xs2_shard[lo * 128:hi * 128, :].opt()],
                        outs=[xs2q_int[q][:].opt()],
                    )

            edge_layer([t[:] for t in xs1_q], xsb[:], s1_t, t1_t,
                       post_group_a)

            lg = res.tile([128, 2 * G], fp32)

            def post_group_b(g, hblk):
                plg = ps.tile([128, 2], fp32, name="plg", tag="plg",
                              bufs=2)
                nc.tensor.matmul(plg[:], hblk[:], wf_t[:], start=True,
                                 stop=True)
                nc.vector.tensor_add(out=lg[:, 2 * g:2 * g + 2],
                                     in0=plg[:], in1=bf_t[:])

            edge_layer([t[:] for t in xs2q_int], xs2b[:], s2_t, t2_t,
                       post_group_b)

            def strided(base, start):
                a = base[:]
                return bass.AP(a.tensor, a.offset + start,
                               [a.ap[0], [2, G]])

            z0, z1 = strided(lg, 0), strided(lg, 1)
            mx = res.tile([128, G], fp32)
            nc.vector.tensor_tensor(out=mx[:], in0=z0, in1=z1,
                                    op=mybir.AluOpType.max)
            sm0 = res.tile([128, G], fp32)
            sm1 = res.tile([128, G], fp32)
            nc.vector.tensor_sub(out=sm0[:], in0=z0, in1=mx[:])
            nc.vector.tensor_sub(out=sm1[:], in0=z1, in1=mx[:])
            e0 = res.tile([128, G], fp32)
            e1 = res.tile([128, G], fp32)
            nc.scalar.activation(out=e0[:], in_=sm0[:], func=AF.Exp)
            nc.scalar.activation(out=e1[:], in_=sm1[:], func=AF.Exp)
            se = res.tile([128, G], fp32)
            nc.vector.tensor_add(out=se[:], in0=e0[:], in1=e1[:])
            ls = res.tile([128, G], fp32)
            nc.scalar.activation(out=ls[:], in_=se[:], func=AF.Ln)
            nc.vector.tensor_sub(out=sm0[:], in0=sm0[:], in1=ls[:])
            nc.vector.tensor_sub(out=sm1[:], in0=sm1[:], in1=ls[:])
            lpo = res.tile([128, 2 * G], fp32)
            nc.vector.tensor_copy(out=strided(lpo, 0), in_=sm0[:])
            nc.vector.tensor_copy(out=strided(lpo, 1), in_=sm1[:])
            nc.sync.dma_start(out=out_lp[:], in_=lpo[:])

    nc.compile()
    return nc


# ---------------------------------------------------------------- main entry
def _run(x, edge_index, game_indices,
         W1, b1, g1, be1, m1, v1, W2, b2, g2, be2, m2, v2, Wf, bf,
         trace=False, cfg=None):
    from concourse import bass_utils

    if cfg is None:
        cfg = dict(N=N, NPAD=NPAD, SHARD=SHARD, NC=NC, H=H, F_IN=F_IN,
                   NCHUNK=NCHUNK, COH=COH, NQ=NQ, NIDX_CAP=1024)

    x = np.asarray(x, dtype=np.float32)
    key = ("prep", x.shape, int(np.asarray(edge_index)[0, 0]),
           int(np.asarray(edge_index).sum() % (1 << 31)))
    if key in _CACHE:
        per_core, meta, G = _CACHE[key]
    else:
        per_core, meta, G = _prepare(x, np.asarray(edge_index), cfg)
        # colq: map call col_lo -> chunk q (for table selection)
        colq = {}
        ncoh = len(meta["call_meta"])
        coh = cfg["COH"]
        col = 0
        # reproduce prep's chunk-major col layout
        import numpy as _np
        C_gq = _np.zeros((G, cfg["NCHUNK"]), dtype=_np.int64)
        for g in range(G):
            for (q, off, cgq, mt0, ntl) in meta["grp_meta"][g]:
                C_gq[g, q] += cgq
        for h in range(ncoh):
            gs = list(range(h * coh, min((h + 1) * coh, G)))
            for q in range(cfg["NCHUNK"]):
                R = int(C_gq[gs, q].sum())
                T = (R + 127) // 128
                for t0 in range(0, T, cfg["NIDX_CAP"] // 128):
                    colq[col + t0 * 8] = q
                col += T * 8
        meta["colq"] = colq
        _CACHE.clear()
        _CACHE[key] = (per_core, meta, G)

    bkey = ("bass", G, meta["total_tiles"], meta["n_oh"],
            tuple(tuple(m) for h_ in meta["call_meta"] for m in h_))
    if bkey in _CACHE:
        nc_m = _CACHE[bkey]
    else:
        nc_m = _build(cfg, G, meta)
        _CACHE[bkey] = nc_m

    s1, t1 = _fold_bn(np.asarray(g1), np.asarray(be1), np.asarray(m1),
                      np.asarray(v1), np.asarray(b1))
    s2, t2 = _fold_bn(np.asarray(g2), np.asarray(be2), np.asarray(m2),
                      np.asarray(v2), np.asarray(b2))
    Tg_max = max(meta["oh_tiles"])
    iota = np.tile(np.arange(128, dtype=np.float16), (128, Tg_max))
    iden = np.eye(128, dtype=np.float16)
    bf_rep = np.broadcast_to(np.asarray(bf, dtype=np.float32), (128, 2)).copy()

    ncores = cfg["NC"]
    in_maps = []
    for c in range(ncores):
        pc = per_core[c]
        in_maps.append(dict(
            xT=pc["xT"], W1=np.asarray(W1, np.float16),
            W2=np.asarray(W2, np.float16), Wf=np.asarray(Wf, np.float16),
            bf_rep=bf_rep, s1=s1, t1=t1, s2=s2, t2=t2, iota=iota,
            iden=iden, idx1=pc["idx16"], dloc1=pc["dloc"],
            ddrow=pc["ddrow"], dinv_nat=pc["dinv_nat"],
        ))
    res = bass_utils.run_bass_kernel_spmd(
        nc_m, in_maps, core_ids=list(range(ncores)), trace=trace)

    class _Res:
        pass

    r = _Res()
    r.results = res.results
    r.exec_time_ns = res.exec_time_ns
    r.parts = (res,)

    gi = np.asarray(game_indices, dtype=np.int64)
    shard = cfg["SHARD"]
    ci = gi // shard
    gidx = (gi % shard) // 128
    pi = gi % 128
    lp = np.stack([res.results[c]["logp"] for c in range(ncores)])
    out = np.empty((gi.shape[0], 2), dtype=np.float32)
    out[:, 0] = lp[ci, pi, 2 * gidx]
    out[:, 1] = lp[ci, pi, 2 * gidx + 1]
    return out, r


def kernel(**inputs):
    out, _ = _run(**inputs)
    return out


def kernel_profiled(**inputs):
    out, res = _run(**inputs, trace=True)
    return out, res
